# revision 28
# baseline (speedup 1.0000x reference)
"""Trainium2 Bass kernel for a 2-layer GAT + global mean pool + linear head.

Strategy (8 NeuronCores, SPMD single program, per-core data):
  - Nodes are partitioned into 8 contiguous shards of 1250; each core owns the
    edges whose dst falls in its shard (grouped by dst block of 128, sorted).
  - D1 is SHARDED: each core computes h1 = x_shard @ W1 for its 1250 nodes
    (als/ald logits folded into the matmul via host precompute), writes a
    node-major row table [nloc, 640] bf16 (cols 0:512 = h1, 512:516 = als),
    then an 8-rank AllGather (Shared scratchpad output) builds the full
    gather table [N, 640].  Dst logits (ald) stay in SBUF.
  - Edge phase: per dst-block, NON-self-loop edge source rows are fetched
    with dma_gather (SWDGE; cost ~8ns/idx, so self-loops - whose rows are
    the block's own contiguous shard rows - are handled by a direct DMA +
    vector path instead, and per-block tile counts are trimmed to the max
    actual edge count across cores).  Per-edge dst logits come from the
    tensor engine as segT_tile^T @ ald_block.  Segment-softmax runs without
    max-subtraction; normalization after aggregation via seg matmuls in PSUM.
  - D2 is interleaved into the E1 block loop; layer 2 exchange via a second
    AllGather; pooled partials with an AllReduce.
"""
import os
import sys
import numpy as np

for _p in ("/opt/trn_rl_repo", "/root/.axon_site/_ro/trn_rl_repo"):
    if os.path.isdir(_p) and _p not in sys.path:
        sys.path.append(_p)

import ml_dtypes

BF16 = ml_dtypes.bfloat16

# -------- problem constants (hardcoded per contest rules) --------
N = 10000
E = 160000
F_IN = 768
H1 = 4
C = 128
OUT = 10
G = 16
NEG_SLOPE = 0.2
P = 128
N_CORES = 8
KC1 = F_IN // P          # 6 k-chunks for layer-1 matmul
KC2 = (H1 * C) // P      # 4 k-chunks for layer-2 matmul
ROW1 = 640               # h-pack row: 512 h | 4 als | 124 pad   (1280 B bf16)
ROW2 = 256               # h2-pack row: 128 h2 | 1 als2 | 127 pad (512 B)


def _bf(x):
    return np.ascontiguousarray(np.asarray(x, dtype=np.float32).astype(BF16))


def _prep(x, edge_index, batch, W1, att_src1, att_dst1, b1, W2, att_src2,
          att_dst2, b2, Wc, bc, n_cores=N_CORES):
    """Host-side index/layout preprocessing. Returns (common, per_core, meta)."""
    x = np.asarray(x, np.float32)
    edge_index = np.asarray(edge_index, np.int64)
    batch = np.asarray(batch, np.int64)
    nloc = N // n_cores
    nblk = (nloc + P - 1) // P
    # self-loops (one per node, appended by the reference) are handled by a
    # dense local path on device; only the raw edges go through the gather.
    src = edge_index[0]
    dst = edge_index[1]

    W1 = np.asarray(W1, np.float32)
    W2 = np.asarray(W2, np.float32)
    W1r = W1.reshape(F_IN, H1, C)
    A1 = np.concatenate([
        np.einsum('khc,hc->kh', W1r, np.asarray(att_src1, np.float32)),
        np.einsum('khc,hc->kh', W1r, np.asarray(att_dst1, np.float32)),
    ], axis=1)                                  # [768, 8]
    A2 = np.stack([W2 @ np.asarray(att_src2, np.float32)[0],
                   W2 @ np.asarray(att_dst2, np.float32)[0]], axis=1)  # [512, 2]

    cnt = np.bincount(batch, minlength=G).astype(np.float32)
    inv_cnt = 1.0 / np.maximum(cnt, 1.0)

    # per-core edge grouping by dst block
    core_blocks = []
    blk_cnt = np.zeros((n_cores, nblk), dtype=np.int64)
    for c in range(n_cores):
        lo = c * nloc
        m = (dst >= lo) & (dst < lo + nloc)
        s_c, d_c = src[m], dst[m] - lo
        order = np.argsort(d_c, kind='stable')
        s_c, d_c = s_c[order], d_c[order]
        blocks = []
        for b in range(nblk):
            bm = (d_c >= b * P) & (d_c < min((b + 1) * P, nloc))
            blocks.append((s_c[bm], d_c[bm] - b * P))
            blk_cnt[c, b] = bm.sum()
        core_blocks.append(blocks)
    # per-block tile count: max edges across cores, ceil-div 128
    tb = tuple(int(v) for v in
               (blk_cnt.max(axis=0) + P - 1) // P)
    ttot = int(sum(tb))

    def idx_wrap(vals):
        v = np.asarray(vals, dtype=np.int16)
        out = np.zeros((16, len(v) // 16), dtype=np.int16)
        i = np.arange(len(v))
        out[i % 16, i // 16] = v
        return np.tile(out, (8, 1))

    common = dict(
        w1=_bf(W1), a1=_bf(A1), w2=_bf(W2), a2=_bf(A2),
        wc=np.ascontiguousarray(np.asarray(Wc, np.float32)),
        bcb=np.ascontiguousarray(np.tile(np.asarray(bc, np.float32), (G, 1))),
        b1b=np.ascontiguousarray(np.tile(np.asarray(b1, np.float32), (P, 1))),
        b2b=np.ascontiguousarray(np.tile(np.asarray(b2, np.float32), (P, 1))),
    )

    per_core = []
    for c in range(n_cores):
        lo = c * nloc
        srccols = []
        seg = np.zeros((ttot, P, P), dtype=BF16)
        toff = 0
        for b in range(nblk):
            sb, db = core_blocks[c][b]
            ne = tb[b] * P
            s_pad = np.zeros(ne, dtype=np.int64); s_pad[:len(sb)] = sb
            srccols.append(idx_wrap(s_pad))
            ei = np.arange(len(db))
            seg[toff + ei // P, ei % P, db] = BF16(1.0)
            toff += tb[b]
        poolm = np.zeros((nblk * P, G), dtype=np.float32)
        gg = batch[lo:lo + nloc]
        poolm[np.arange(nloc), gg] = inv_cnt[gg]
        per_core.append(dict(
            xTloc=_bf(x[lo:lo + nloc].T),       # [768, nloc]
            srcidx=np.ascontiguousarray(np.concatenate(srccols, axis=1)),
            # seg shipped pre-transposed: [128 (edge), ttot * 128 (tile, dst)]
            seg=np.ascontiguousarray(seg.transpose(1, 0, 2).reshape(P, ttot * P)),
            # segT: [128 (dst), ttot * 128 (tile, edge)]
            segT=np.ascontiguousarray(seg.transpose(2, 0, 1).reshape(P, ttot * P)),
            # poolm shipped pre-transposed: [128 (node-in-chunk), nblk * 16]
            poolm=np.ascontiguousarray(
                poolm.reshape(nblk, P, G).transpose(1, 0, 2).reshape(P, nblk * G)),
        ))
    meta = dict(n_cores=n_cores, nloc=nloc, nblk=nblk, tb=tb)
    return common, per_core, meta


# ------------------------------------------------------------------
#  device program
# ------------------------------------------------------------------

def _build(meta, phases='full'):
    from concourse import bass, bacc, tile, mybir
    from concourse.masks import make_identity

    n_cores, nloc, nblk, tb = (meta['n_cores'], meta['nloc'],
                               meta['nblk'], meta['tb'])
    ttot = sum(tb)
    t_max = max(tb)
    toffs = [sum(tb[:b]) for b in range(nblk)]
    ncols = ttot * 8                      # idx tensor cols
    bf16, f32, i16 = mybir.dt.bfloat16, mybir.dt.float32, mybir.dt.int16

    nc = bacc.Bacc("TRN2", target_bir_lowering=False, debug=False,
                   num_devices=n_cores)

    # ---- I/O ----
    d_xTloc = nc.dram_tensor("xTloc", [F_IN, nloc], bf16, kind="ExternalInput")
    d_w1 = nc.dram_tensor("w1", [F_IN, 512], bf16, kind="ExternalInput")
    d_a1 = nc.dram_tensor("a1", [F_IN, 8], bf16, kind="ExternalInput")
    d_w2 = nc.dram_tensor("w2", [512, C], bf16, kind="ExternalInput")
    d_a2 = nc.dram_tensor("a2", [512, 2], bf16, kind="ExternalInput")
    d_wc = nc.dram_tensor("wc", [C, OUT], f32, kind="ExternalInput")
    d_bcb = nc.dram_tensor("bcb", [G, OUT], f32, kind="ExternalInput")
    d_b1b = nc.dram_tensor("b1b", [P, 512], f32, kind="ExternalInput")
    d_b2b = nc.dram_tensor("b2b", [P, C], f32, kind="ExternalInput")
    d_srci = nc.dram_tensor("srcidx", [P, ncols], i16, kind="ExternalInput")
    d_seg = nc.dram_tensor("seg", [P, ttot * P], bf16, kind="ExternalInput")
    d_segT = nc.dram_tensor("segT", [P, ttot * P], bf16, kind="ExternalInput")
    d_poolm = nc.dram_tensor("poolm", [P, nblk * G], f32, kind="ExternalInput")
    d_out = nc.dram_tensor("out", [G, OUT], f32, kind="ExternalOutput")

    with tile.TileContext(nc) as tc:
        with tc.tile_pool(name="dram", bufs=1, space="DRAM") as dram, \
             tc.tile_pool(name="const", bufs=1) as const, \
             tc.tile_pool(name="persist", bufs=1) as persist, \
             tc.tile_pool(name="psU", bufs=2, space="PSUM") as psU, \
             tc.tile_pool(name="psD", bufs=2, space="PSUM") as psD, \
             tc.tile_pool(name="psT", bufs=2, space="PSUM") as psT, \
             tc.tile_pool(name="psA", bufs=1, space="PSUM") as psA, \
             tc.tile_pool(name="psP", bufs=1, space="PSUM") as psP, \
             tc.tile_pool(name="work", bufs=2) as work, \
             tc.tile_pool(name="rows", bufs=3) as rows:

            # ---- DRAM internals ----
            h_shard = dram.tile([nloc, ROW1], bf16)      # local D1 rows
            h_pack = dram.tile([N, ROW1], bf16, addr_space="Shared")
            h2_in = dram.tile([nloc, ROW2], bf16)        # AllGather send shard
            h2_full = dram.tile([N, ROW2], bf16, addr_space="Shared")
            pool_in = dram.tile([P, G], f32)
            pool_out = dram.tile([P, G], f32)
            pool_in2 = dram.tile([P, G], f32)
            pool_out2 = dram.tile([P, G], f32)

            # ---- SBUF constants needed by D1 (loaded first) ----
            w1_sb = const.tile([P, KC1, 512], bf16)
            nc.sync.dma_start(out=w1_sb[:], in_=d_w1.ap().rearrange("(kc p) n -> p kc n", p=P))
            a1_sb = const.tile([P, KC1, 8], bf16)
            nc.sync.dma_start(out=a1_sb[:], in_=d_a1.ap().rearrange("(kc p) n -> p kc n", p=P))

            h1T_sb = persist.tile([P, KC2, nblk * P], bf16)  # relu(out1) transposed
            pald_all = persist.tile([P, sum(tb), 4], f32)  # per-edge dst logits L1
            pald2_all = persist.tile([P, sum(tb), 1], f32)
            ald_sb = persist.tile([P, nblk, 4], bf16)     # local dst logits L1
            ald2_sb = persist.tile([P, nblk, 1], bf16)    # local dst logits L2
            # rows past nd of the last block feed matmuls (x0) - keep finite
            nc.gpsimd.memset(ald_sb[:], 0.0)
            nc.gpsimd.memset(ald2_sb[:], 0.0)

            do_e1 = phases in ('d1e1', 'd1e1d2', 'nocoll', 'full')
            do_d2 = phases in ('d1e1d2', 'nocoll', 'full')
            do_e2 = phases in ('nocoll', 'full')
            do_coll = phases == 'full'

            # ====== D1: h1 = x_shard @ W1 (local nodes only, sharded) =======
            with tc.tile_pool(name="xpool", bufs=1) as xpool:
                xg = xpool.tile([P, KC1, nloc], bf16, name="xg")
                nc.sync.dma_start(
                    out=xg[:], in_=d_xTloc.ap().rearrange("(kc p) n -> p kc n", p=P))
                for j in range(nblk):
                    nd = min(P, nloc - j * P)
                    ph = psU.tile([P, 512], f32, tag="U")
                    pal = psD.tile([P, 8], f32, tag="den")
                    for kc in range(KC1):
                        lhs = xg[:, kc, j * P: j * P + nd]
                        nc.tensor.matmul(out=ph[0:nd, :], lhsT=lhs,
                                         rhs=w1_sb[:, kc, :],
                                         start=(kc == 0), stop=(kc == KC1 - 1))
                        nc.tensor.matmul(out=pal[0:nd, :], lhsT=lhs,
                                         rhs=a1_sb[:, kc, :],
                                         start=(kc == 0), stop=(kc == KC1 - 1))
                    hrow = rows.tile([P, ROW1], bf16, tag="hrow", bufs=2)
                    nc.gpsimd.memset(hrow[0:nd, 516:ROW1], 0.0)
                    nc.vector.tensor_copy(out=hrow[0:nd, 0:512], in_=ph[0:nd, :])
                    nc.vector.tensor_copy(out=hrow[0:nd, 512:516], in_=pal[0:nd, 0:4])
                    nc.sync.dma_start(out=h_shard[j * P: j * P + nd, :],
                                      in_=hrow[0:nd, :])
                    nc.vector.tensor_copy(out=ald_sb[0:nd, j, :], in_=pal[0:nd, 4:8])

            # ---- exchange layer-1 gather table ----
            if do_coll:
                nc.gpsimd.collective_compute(
                    "AllGather", mybir.AluOpType.bypass,
                    replica_groups=[list(range(n_cores))],
                    ins=[h_shard.opt()], outs=[h_pack.opt()])
            else:
                nc.sync.dma_start(out=h_pack[0:nloc, :], in_=h_shard[:, :])


            # tile index -> owning block
            tile_blk = []
            for _b in range(nblk):
                tile_blk += [_b] * tb[_b]

            # ---- pald (L1 per-edge dst logits) streamed during AllGather ----
            CH = 32
            with tc.tile_pool(name="segs", bufs=2) as segs:
                for g0 in range(0, ttot, CH):
                    nt = min(CH, ttot - g0)
                    sT = segs.tile([P, CH, P], bf16, tag="sT")
                    nc.sync.dma_start(
                        out=sT[:, 0:nt, :],
                        in_=d_segT.ap().rearrange("p (t q) -> p t q", q=P)[:, g0:g0 + nt, :])
                    pch = psA.tile([P, CH, 4], f32, tag="pald")
                    for k in range(nt):
                        nc.tensor.matmul(out=pch[:, k, :], lhsT=sT[:, k, :],
                                         rhs=ald_sb[:, tile_blk[g0 + k], :],
                                         start=True, stop=True)
                    nc.vector.tensor_copy(out=pald_all[:, g0:g0 + nt, :],
                                          in_=pch[:, 0:nt, :])

            # ---- remaining SBUF constants (needed from E1 on) ----
            w2_sb = const.tile([P, KC2, C], bf16)
            nc.sync.dma_start(out=w2_sb[:], in_=d_w2.ap().rearrange("(kc p) n -> p kc n", p=P))
            a2_sb = const.tile([P, KC2, 2], bf16)
            nc.sync.dma_start(out=a2_sb[:], in_=d_a2.ap().rearrange("(kc p) n -> p kc n", p=P))
            wc_sb = const.tile([P, OUT], f32)
            nc.sync.dma_start(out=wc_sb[:], in_=d_wc[:, :])
            bcb_sb = const.tile([G, OUT], f32)
            nc.sync.dma_start(out=bcb_sb[:], in_=d_bcb[:, :])
            b1b_sb = const.tile([P, 512], f32)
            nc.sync.dma_start(out=b1b_sb[:], in_=d_b1b[:, :])
            b2b_sb = const.tile([P, C], f32)
            nc.sync.dma_start(out=b2b_sb[:], in_=d_b2b[:, :])
            srci_sb = const.tile([P, ncols], i16)
            nc.sync.dma_start(out=srci_sb[:], in_=d_srci[:, :])
            poolm_sb = const.tile([P, nblk, G], f32)
            nc.sync.dma_start(out=poolm_sb[:], in_=d_poolm.ap())
            ident = const.tile([P, P], bf16)
            make_identity(nc, ident[:])

            def emit_d2(j):
                nd2 = min(P, nloc - j * P)
                o1b_j, _ = o1bs[j]
                for kc in range(KC2):
                    tp = psT.tile([P, P], bf16, tag="tp")
                    nc.tensor.transpose(out=tp[:, 0:nd2],
                                        in_=o1b_j[0:nd2, kc * P:(kc + 1) * P],
                                        identity=ident[0:nd2, 0:nd2])
                    nc.scalar.activation(h1T_sb[:, kc, j * P: j * P + nd2],
                                         tp[:, 0:nd2],
                                         mybir.ActivationFunctionType.Copy)
                p2 = psU.tile([P, C], f32, tag="U")
                p2a = psD.tile([P, 2], f32, tag="den")
                for kc in range(KC2):
                    lhs = h1T_sb[:, kc, j * P: j * P + nd2]
                    nc.tensor.matmul(out=p2[0:nd2, :], lhsT=lhs, rhs=w2_sb[:, kc, :],
                                     start=(kc == 0), stop=(kc == KC2 - 1))
                    nc.tensor.matmul(out=p2a[0:nd2, :], lhsT=lhs, rhs=a2_sb[:, kc, :],
                                     start=(kc == 0), stop=(kc == KC2 - 1))
                r2 = rows.tile([P, ROW2], bf16, tag="r2", name="r2")
                nc.gpsimd.memset(r2[0:nd2, C + 1:ROW2], 0.0)
                nc.scalar.activation(r2[0:nd2, 0:C], p2[0:nd2, :],
                                     mybir.ActivationFunctionType.Copy)
                nc.scalar.activation(r2[0:nd2, C:C + 1], p2a[0:nd2, 0:1],
                                     mybir.ActivationFunctionType.Copy)
                nc.sync.dma_start(out=h2_in[j * P: j * P + nd2, :], in_=r2[0:nd2, :])
                nc.scalar.activation(ald2_sb[0:nd2, j, :], p2a[0:nd2, 1:2],
                                     mybir.ActivationFunctionType.Copy)

            o1bs = {}
            # ========== E1 (+ D2 of block b-1, whose deps are ready) ========
            for b in range(nblk if do_e1 else 0):
                nd = min(P, nloc - b * P)
                t_b = tb[b]
                to = toffs[b]
                nidx = t_b * P
                cb = to * 8
                hg = work.tile([P, t_max, ROW1], bf16, tag="hg", bufs=5)
                nc.gpsimd.dma_gather(hg[:, 0:t_b, :], h_pack[:, :],
                                     srci_sb[:, cb:cb + t_b * 8],
                                     nidx, nidx, ROW1, single_packet=False)
                segc = work.tile([P, t_max, P], bf16, tag="segc", bufs=3)
                nc.sync.dma_start(
                    out=segc[:, 0:t_b, :],
                    in_=d_seg.ap().rearrange("p (t q) -> p t q", q=P)[:, to:to + t_b, :])

                if do_d2 and b > 0:
                    emit_d2(b - 1)

                # self-loop row block: local shard rows (contiguous)
                hself = work.tile([P, 516], bf16, tag="hself")
                nc.sync.dma_start(out=hself[0:nd, :],
                                  in_=h_shard[b * P: b * P + nd, 0:516])

                s32 = work.tile([P, t_max, 4], f32, tag="s32")
                nc.vector.tensor_tensor(out=s32[:, 0:t_b, :],
                                        in0=hg[:, 0:t_b, 512:516],
                                        in1=pald_all[:, to:to + t_b, :],
                                        op=mybir.AluOpType.add)
                ea = work.tile([P, t_max, 4], f32, tag="ea")
                nc.scalar.activation(ea[:, 0:t_b, :], s32[:, 0:t_b, :],
                                     mybir.ActivationFunctionType.Exp)
                eb = work.tile([P, t_max, 4], f32, tag="eb")
                nc.scalar.activation(eb[:, 0:t_b, :], s32[:, 0:t_b, :],
                                     mybir.ActivationFunctionType.Exp,
                                     scale=NEG_SLOPE)
                pbf = work.tile([P, t_max, 4], bf16, tag="pbf")
                nc.vector.tensor_tensor(out=pbf[:, 0:t_b, :], in0=ea[:, 0:t_b, :],
                                        in1=eb[:, 0:t_b, :], op=mybir.AluOpType.max)

                # self-loop attention: p_self = exp(leaky(als + ald))
                sself = work.tile([P, 4], f32, tag="sself")
                nc.vector.tensor_tensor(out=sself[0:nd, :], in0=hself[0:nd, 512:516],
                                        in1=ald_sb[0:nd, b, :], op=mybir.AluOpType.add)
                eaself = work.tile([P, 4], f32, tag="eaself")
                nc.scalar.activation(eaself[0:nd, :], sself[0:nd, :],
                                     mybir.ActivationFunctionType.Exp)
                ebself = work.tile([P, 4], f32, tag="ebself")
                nc.scalar.activation(ebself[0:nd, :], sself[0:nd, :],
                                     mybir.ActivationFunctionType.Exp,
                                     scale=NEG_SLOPE)
                pself = work.tile([P, 4], f32, tag="pself")
                nc.vector.tensor_tensor(out=pself[0:nd, :], in0=eaself[0:nd, :],
                                        in1=ebself[0:nd, :], op=mybir.AluOpType.max)

                th = t_b // 2
                hgA = hg[:, 0:th, 0:512].rearrange("p t (h c) -> p t h c", h=H1)
                nc.vector.tensor_tensor(
                    out=hgA, in0=hgA,
                    in1=pbf[:, 0:th, :].unsqueeze(3).to_broadcast([P, th, H1, C]),
                    op=mybir.AluOpType.mult)
                hgB = hg[:, th:t_b, 0:512].rearrange("p t (h c) -> p t h c", h=H1)
                nc.vector.tensor_tensor(
                    out=hgB, in0=hgB,
                    in1=pbf[:, th:t_b, :].unsqueeze(3).to_broadcast([P, t_b - th, H1, C]),
                    op=mybir.AluOpType.mult)

                den = psD.tile([P, 4], f32, tag="den")
                for t in range(t_b):
                    nc.tensor.matmul(out=den[:, :], lhsT=segc[:, t, :],
                                     rhs=pbf[:, t, :],
                                     start=(t == 0), stop=(t == t_b - 1))
                U = psU.tile([P, 512], f32, tag="U")
                for t in range(t_b):
                    nc.tensor.matmul(out=U[:, :], lhsT=segc[:, t, :],
                                     rhs=hg[:, t, 0:512],
                                     start=(t == 0), stop=(t == t_b - 1))

                # add self-loop contribution, normalize, bias, relu
                o1 = work.tile([P, 512], f32, tag="o1")
                nc.vector.tensor_tensor(
                    out=o1[0:nd, :].rearrange("p (h c) -> p h c", h=H1),
                    in0=hself[0:nd, 0:512].rearrange("p (h c) -> p h c", h=H1),
                    in1=pself[0:nd, :].unsqueeze(2).to_broadcast([nd, H1, C]),
                    op=mybir.AluOpType.mult)
                dtot = work.tile([P, 4], f32, tag="dtot")
                nc.vector.tensor_tensor(out=dtot[0:nd, :], in0=den[0:nd, :],
                                        in1=pself[0:nd, :], op=mybir.AluOpType.add)
                rec = work.tile([P, 4], f32, tag="rec")
                nc.vector.reciprocal(rec[0:nd, :], dtot[0:nd, :])
                nc.vector.tensor_tensor(out=o1[0:nd, :], in0=U[0:nd, :],
                                        in1=o1[0:nd, :], op=mybir.AluOpType.add)
                o1h = o1[0:nd, :].rearrange("p (h c) -> p h c", h=H1)
                nc.vector.tensor_tensor(
                    out=o1h, in0=o1h,
                    in1=rec[0:nd, :].unsqueeze(2).to_broadcast([nd, H1, C]),
                    op=mybir.AluOpType.mult)
                nc.vector.tensor_tensor(out=o1[0:nd, :], in0=o1[0:nd, :],
                                        in1=b1b_sb[0:nd, :], op=mybir.AluOpType.add)
                o1b = work.tile([P, 512], bf16, tag="o1b", bufs=2)
                nc.scalar.activation(o1b[0:nd, :], o1[0:nd, :],
                                     mybir.ActivationFunctionType.Relu)
                o1bs[b] = (o1b, nd)

            # ---- D2 for the final block ----
            if do_d2:
                emit_d2(nblk - 1)

            # ---- exchange layer-2 inputs ----
            if do_coll:
                nc.gpsimd.collective_compute(
                    "AllGather", mybir.AluOpType.bypass,
                    replica_groups=[list(range(n_cores))],
                    ins=[h2_in.opt()], outs=[h2_full.opt()])
            elif do_e2:
                nc.sync.dma_start(out=h2_full[0:nloc, :], in_=h2_in[:, :])

            # ---- pald2 streamed during the second AllGather ----
            with tc.tile_pool(name="segs2", bufs=2) as segs2:
                for g0 in range(0, ttot, CH):
                    nt = min(CH, ttot - g0)
                    sT2 = segs2.tile([P, CH, P], bf16, tag="sT2")
                    nc.sync.dma_start(
                        out=sT2[:, 0:nt, :],
                        in_=d_segT.ap().rearrange("p (t q) -> p t q", q=P)[:, g0:g0 + nt, :])
                    pch2 = psA.tile([P, CH, 4], f32, tag="pald")
                    for k in range(nt):
                        nc.tensor.matmul(out=pch2[:, k, 0:1], lhsT=sT2[:, k, :],
                                         rhs=ald2_sb[:, tile_blk[g0 + k], :],
                                         start=True, stop=True)
                    nc.vector.tensor_copy(out=pald2_all[:, g0:g0 + nt, :],
                                          in_=pch2[:, 0:nt, 0:1])

            # ================= E2: layer-2 edge phase =======================
            poolT = psP.tile([P, G], f32, tag="poolT")
            for b in range(nblk if do_e2 else 0):
                nd = min(P, nloc - b * P)
                t_b = tb[b]
                to = toffs[b]
                nidx = t_b * P
                cb = to * 8
                hg2 = work.tile([P, t_max, ROW2], bf16, tag="hg2", bufs=3)
                nc.gpsimd.dma_gather(hg2[:, 0:t_b, :], h2_full[:, :],
                                     srci_sb[:, cb:cb + t_b * 8],
                                     nidx, nidx, ROW2, single_packet=False)
                if do_coll and b == nblk - 1:
                    poolTa_sb = work.tile([P, G], f32, tag="poolTa_sb")
                    nc.scalar.activation(poolTa_sb[:], poolT[:, :],
                                         mybir.ActivationFunctionType.Copy)
                    nc.sync.dma_start(out=pool_in[:, :], in_=poolTa_sb[:])
                    nc.gpsimd.collective_compute(
                        "AllReduce", mybir.AluOpType.add,
                        replica_groups=[list(range(n_cores))],
                        ins=[pool_in.opt()], outs=[pool_out.opt()])
                segc2 = work.tile([P, t_max, P], bf16, tag="segc", bufs=3)
                nc.sync.dma_start(
                    out=segc2[:, 0:t_b, :],
                    in_=d_seg.ap().rearrange("p (t q) -> p t q", q=P)[:, to:to + t_b, :])

                h2self = work.tile([P, C + 1], bf16, tag="h2self")
                nc.sync.dma_start(out=h2self[0:nd, :],
                                  in_=h2_in[b * P: b * P + nd, 0:C + 1])

                s2 = work.tile([P, t_max, 1], f32, tag="s2")
                nc.vector.tensor_tensor(out=s2[:, 0:t_b, :], in0=hg2[:, 0:t_b, C:C + 1],
                                        in1=pald2_all[:, to:to + t_b, :],
                                        op=mybir.AluOpType.add)
                ea2 = work.tile([P, t_max, 1], f32, tag="ea2")
                nc.scalar.activation(ea2[:, 0:t_b, :], s2[:, 0:t_b, :],
                                     mybir.ActivationFunctionType.Exp)
                eb2 = work.tile([P, t_max, 1], f32, tag="eb2")
                nc.scalar.activation(eb2[:, 0:t_b, :], s2[:, 0:t_b, :],
                                     mybir.ActivationFunctionType.Exp,
                                     scale=NEG_SLOPE)
                pbf2 = work.tile([P, t_max, 1], bf16, tag="pbf2")
                nc.vector.tensor_tensor(out=pbf2[:, 0:t_b, :], in0=ea2[:, 0:t_b, :],
                                        in1=eb2[:, 0:t_b, :], op=mybir.AluOpType.max)

                s2self = work.tile([P, 1], f32, tag="s2self")
                nc.vector.tensor_tensor(out=s2self[0:nd, :], in0=h2self[0:nd, C:C + 1],
                                        in1=ald2_sb[0:nd, b, :], op=mybir.AluOpType.add)
                ea2self = work.tile([P, 1], f32, tag="ea2self")
                nc.scalar.activation(ea2self[0:nd, :], s2self[0:nd, :],
                                     mybir.ActivationFunctionType.Exp)
                eb2self = work.tile([P, 1], f32, tag="eb2self")
                nc.scalar.activation(eb2self[0:nd, :], s2self[0:nd, :],
                                     mybir.ActivationFunctionType.Exp,
                                     scale=NEG_SLOPE)
                p2self = work.tile([P, 1], f32, tag="p2self")
                nc.vector.tensor_tensor(out=p2self[0:nd, :], in0=ea2self[0:nd, :],
                                        in1=eb2self[0:nd, :], op=mybir.AluOpType.max)

                nc.vector.tensor_tensor(
                    out=hg2[:, 0:t_b, 0:C], in0=hg2[:, 0:t_b, 0:C],
                    in1=pbf2[:, 0:t_b, 0:1].to_broadcast([P, t_b, C]),
                    op=mybir.AluOpType.mult)

                den2 = psD.tile([P, 1], f32, tag="den")
                for t in range(t_b):
                    nc.tensor.matmul(out=den2[:, :], lhsT=segc2[:, t, :],
                                     rhs=pbf2[:, t, :],
                                     start=(t == 0), stop=(t == t_b - 1))
                U2 = psU.tile([P, C], f32, tag="U")
                for t in range(t_b):
                    nc.tensor.matmul(out=U2[:, :], lhsT=segc2[:, t, :],
                                     rhs=hg2[:, t, 0:C],
                                     start=(t == 0), stop=(t == t_b - 1))

                o2self = work.tile([P, C], f32, tag="o2self")
                nc.vector.tensor_tensor(out=o2self[0:nd, :], in0=h2self[0:nd, 0:C],
                                        in1=p2self[0:nd, 0:1].to_broadcast([nd, C]),
                                        op=mybir.AluOpType.mult)
                d2tot = work.tile([P, 1], f32, tag="d2tot")
                nc.vector.tensor_tensor(out=d2tot[0:nd, :], in0=den2[0:nd, :],
                                        in1=p2self[0:nd, :], op=mybir.AluOpType.add)
                rec2 = work.tile([P, 1], f32, tag="rec2")
                nc.vector.reciprocal(rec2[0:nd, :], d2tot[0:nd, :])
                o2 = work.tile([P, C], f32, tag="o2")
                nc.vector.tensor_tensor(out=o2[0:nd, :], in0=U2[0:nd, :],
                                        in1=o2self[0:nd, :], op=mybir.AluOpType.add)
                nc.vector.tensor_tensor(out=o2[0:nd, :], in0=o2[0:nd, :],
                                        in1=rec2[0:nd, 0:1].to_broadcast([nd, C]),
                                        op=mybir.AluOpType.mult)
                nc.vector.tensor_tensor(out=o2[0:nd, :], in0=o2[0:nd, :],
                                        in1=b2b_sb[0:nd, :], op=mybir.AluOpType.add)
                o2r = work.tile([P, C], f32, tag="o2r")
                nc.scalar.activation(o2r[0:nd, :], o2[0:nd, :],
                                     mybir.ActivationFunctionType.Relu)
                if b < nblk - 1:
                    nc.tensor.matmul(out=poolT[:, :], lhsT=o2r[0:nd, :],
                                     rhs=poolm_sb[0:nd, b, :],
                                     start=(b == 0), stop=(b == nblk - 2))
                else:
                    poolTb = psD.tile([P, G], f32, tag="den")
                    nc.tensor.matmul(out=poolTb[:, :], lhsT=o2r[0:nd, :],
                                     rhs=poolm_sb[0:nd, b, :],
                                     start=True, stop=True)

            # ================= tail: pool exchange + classifier =============
            if not do_e2:
                dummy = work.tile([G, OUT], f32, tag="dummy")
                nc.vector.tensor_copy(out=dummy[:], in_=bcb_sb[:])
                nc.sync.dma_start(out=d_out[:, :], in_=dummy[:])
            else:
              poolT_sb = work.tile([P, G], f32, tag="poolT_sb")
              nc.scalar.activation(poolT_sb[:], poolTb[:, :],
                                   mybir.ActivationFunctionType.Copy)
              nc.sync.dma_start(out=pool_in2[:, :], in_=poolT_sb[:])
              if do_coll:
                  nc.gpsimd.collective_compute(
                      "AllReduce", mybir.AluOpType.add,
                      replica_groups=[list(range(n_cores))],
                      ins=[pool_in2.opt()], outs=[pool_out2.opt()])
              else:
                  nc.sync.dma_start(out=pool_out[:, :], in_=pool_in[:, :])
                  nc.sync.dma_start(out=pool_out2[:, :], in_=pool_in2[:, :])
              poolF_sb = work.tile([P, G], f32, tag="poolF_sb")
              nc.sync.dma_start(out=poolF_sb[:], in_=pool_out[:, :])
              poolF2_sb = work.tile([P, G], f32, tag="poolF2_sb")
              nc.sync.dma_start(out=poolF2_sb[:], in_=pool_out2[:, :])
              ofin = psD.tile([G, OUT], f32, tag="den")
              nc.tensor.matmul(out=ofin[:, :], lhsT=poolF_sb[:], rhs=wc_sb[:],
                               start=True, stop=False)
              nc.tensor.matmul(out=ofin[:, :], lhsT=poolF2_sb[:], rhs=wc_sb[:],
                               start=False, stop=True)
              ofin_sb = work.tile([G, OUT], f32, tag="ofin_sb")
              nc.vector.tensor_tensor(out=ofin_sb[:], in0=ofin[:, :], in1=bcb_sb[:],
                                      op=mybir.AluOpType.add)
              nc.sync.dma_start(out=d_out[:, :], in_=ofin_sb[:])

    nc.compile()
    return nc


# ------------------------------------------------------------------
#  runner
# ------------------------------------------------------------------

_CACHE = {}


def _get_nc(meta):
    key = (meta['n_cores'], meta['nblk'], meta['tb'], meta['nloc'])
    if key not in _CACHE:
        _CACHE[key] = _build(meta)
    return _CACHE[key]


def _in_maps(common, per_core):
    maps = []
    for pc in per_core:
        m = dict(common)
        m.update(pc)
        maps.append(m)
    return maps


def kernel(**inputs) -> np.ndarray:
    common, per_core, meta = _prep(**inputs)
    nc = _get_nc(meta)
    from concourse.bass_utils import run_bass_kernel_spmd
    res = run_bass_kernel_spmd(nc, _in_maps(common, per_core),
                               core_ids=list(range(meta['n_cores'])))
    return np.asarray(res.results[0]['out'], np.float32).reshape(-1)


# revision 29
# speedup vs baseline: 1.0124x; 1.0124x over previous
"""Trainium2 Bass kernel for a 2-layer GAT + global mean pool + linear head.

Strategy (8 NeuronCores, SPMD single program, per-core data):
  - Nodes are partitioned into 8 contiguous shards of 1250; each core owns the
    edges whose dst falls in its shard (grouped by dst block of 128, sorted).
  - D1 is SHARDED: each core computes h1 = x_shard @ W1 for its 1250 nodes
    (als/ald logits folded into the matmul via host precompute), writes a
    node-major row table [nloc, 640] bf16 (cols 0:512 = h1, 512:516 = als),
    then an 8-rank AllGather (Shared scratchpad output) builds the full
    gather table [N, 640].  Dst logits (ald) stay in SBUF.
  - Edge phase: per dst-block, NON-self-loop edge source rows are fetched
    with dma_gather (SWDGE; cost ~8ns/idx, so self-loops - whose rows are
    the block's own contiguous shard rows - are handled by a direct DMA +
    vector path instead, and per-block tile counts are trimmed to the max
    actual edge count across cores).  Per-edge dst logits come from the
    tensor engine as segT_tile^T @ ald_block.  Segment-softmax runs without
    max-subtraction; normalization after aggregation via seg matmuls in PSUM.
  - D2 is interleaved into the E1 block loop; layer 2 exchange via a second
    AllGather; pooled partials with an AllReduce.
"""
import os
import sys
import numpy as np

for _p in ("/opt/trn_rl_repo", "/root/.axon_site/_ro/trn_rl_repo"):
    if os.path.isdir(_p) and _p not in sys.path:
        sys.path.append(_p)

import ml_dtypes

BF16 = ml_dtypes.bfloat16

# -------- problem constants (hardcoded per contest rules) --------
N = 10000
E = 160000
F_IN = 768
H1 = 4
C = 128
OUT = 10
G = 16
NEG_SLOPE = 0.2
P = 128
N_CORES = 8
KC1 = F_IN // P          # 6 k-chunks for layer-1 matmul
KC2 = (H1 * C) // P      # 4 k-chunks for layer-2 matmul
ROW1 = 640               # h-pack row: 512 h | 4 als | 124 pad   (1280 B bf16)
ROW2 = 256               # h2-pack row: 128 h2 | 1 als2 | 127 pad (512 B)


def _bf(x):
    return np.ascontiguousarray(np.asarray(x, dtype=np.float32).astype(BF16))


def _prep(x, edge_index, batch, W1, att_src1, att_dst1, b1, W2, att_src2,
          att_dst2, b2, Wc, bc, n_cores=N_CORES):
    """Host-side index/layout preprocessing. Returns (common, per_core, meta)."""
    x = np.asarray(x, np.float32)
    edge_index = np.asarray(edge_index, np.int64)
    batch = np.asarray(batch, np.int64)
    nloc = N // n_cores
    nblk = (nloc + P - 1) // P
    # self-loops (one per node, appended by the reference) are handled by a
    # dense local path on device; only the raw edges go through the gather.
    src = edge_index[0]
    dst = edge_index[1]

    W1 = np.asarray(W1, np.float32)
    W2 = np.asarray(W2, np.float32)
    W1r = W1.reshape(F_IN, H1, C)
    A1 = np.concatenate([
        np.einsum('khc,hc->kh', W1r, np.asarray(att_src1, np.float32)),
        np.einsum('khc,hc->kh', W1r, np.asarray(att_dst1, np.float32)),
    ], axis=1)                                  # [768, 8]
    A2 = np.stack([W2 @ np.asarray(att_src2, np.float32)[0],
                   W2 @ np.asarray(att_dst2, np.float32)[0]], axis=1)  # [512, 2]

    cnt = np.bincount(batch, minlength=G).astype(np.float32)
    inv_cnt = 1.0 / np.maximum(cnt, 1.0)

    # per-core edge grouping by dst block
    core_blocks = []
    blk_cnt = np.zeros((n_cores, nblk), dtype=np.int64)
    for c in range(n_cores):
        lo = c * nloc
        m = (dst >= lo) & (dst < lo + nloc)
        s_c, d_c = src[m], dst[m] - lo
        order = np.argsort(d_c, kind='stable')
        s_c, d_c = s_c[order], d_c[order]
        blocks = []
        for b in range(nblk):
            bm = (d_c >= b * P) & (d_c < min((b + 1) * P, nloc))
            blocks.append((s_c[bm], d_c[bm] - b * P))
            blk_cnt[c, b] = bm.sum()
        core_blocks.append(blocks)
    # per-block tile count: max edges across cores, ceil-div 128
    tb = tuple(int(v) for v in
               (blk_cnt.max(axis=0) + P - 1) // P)
    ttot = int(sum(tb))

    def idx_wrap(vals):
        v = np.asarray(vals, dtype=np.int16)
        out = np.zeros((16, len(v) // 16), dtype=np.int16)
        i = np.arange(len(v))
        out[i % 16, i // 16] = v
        return np.tile(out, (8, 1))

    common = dict(
        w1=_bf(W1), a1=_bf(A1), w2=_bf(W2), a2=_bf(A2),
        wc=np.ascontiguousarray(np.asarray(Wc, np.float32)),
        bcb=np.ascontiguousarray(np.tile(np.asarray(bc, np.float32), (G, 1))),
        b1b=np.ascontiguousarray(np.tile(np.asarray(b1, np.float32), (P, 1))),
        b2b=np.ascontiguousarray(np.tile(np.asarray(b2, np.float32), (P, 1))),
    )

    per_core = []
    for c in range(n_cores):
        lo = c * nloc
        srccols = []
        seg = np.zeros((ttot, P, P), dtype=BF16)
        toff = 0
        for b in range(nblk):
            sb, db = core_blocks[c][b]
            ne = tb[b] * P
            s_pad = np.zeros(ne, dtype=np.int64); s_pad[:len(sb)] = sb
            srccols.append(idx_wrap(s_pad))
            ei = np.arange(len(db))
            seg[toff + ei // P, ei % P, db] = BF16(1.0)
            toff += tb[b]
        poolm = np.zeros((nblk * P, G), dtype=np.float32)
        gg = batch[lo:lo + nloc]
        poolm[np.arange(nloc), gg] = inv_cnt[gg]
        per_core.append(dict(
            xTloc=_bf(x[lo:lo + nloc].T),       # [768, nloc]
            srcidx=np.ascontiguousarray(np.concatenate(srccols, axis=1)),
            # seg shipped pre-transposed: [128 (edge), ttot * 128 (tile, dst)]
            seg=np.ascontiguousarray(seg.transpose(1, 0, 2).reshape(P, ttot * P)),
            # segT: [128 (dst), ttot * 128 (tile, edge)]
            segT=np.ascontiguousarray(seg.transpose(2, 0, 1).reshape(P, ttot * P)),
            # poolm shipped pre-transposed: [128 (node-in-chunk), nblk * 16]
            poolm=np.ascontiguousarray(
                poolm.reshape(nblk, P, G).transpose(1, 0, 2).reshape(P, nblk * G)),
        ))
    meta = dict(n_cores=n_cores, nloc=nloc, nblk=nblk, tb=tb)
    return common, per_core, meta


# ------------------------------------------------------------------
#  device program
# ------------------------------------------------------------------

def _build(meta, phases='full'):
    from concourse import bass, bacc, tile, mybir
    from concourse.masks import make_identity

    n_cores, nloc, nblk, tb = (meta['n_cores'], meta['nloc'],
                               meta['nblk'], meta['tb'])
    ttot = sum(tb)
    t_max = max(tb)
    toffs = [sum(tb[:b]) for b in range(nblk)]
    ncols = ttot * 8                      # idx tensor cols
    bf16, f32, i16 = mybir.dt.bfloat16, mybir.dt.float32, mybir.dt.int16

    nc = bacc.Bacc("TRN2", target_bir_lowering=False, debug=False,
                   num_devices=n_cores)

    # ---- I/O ----
    d_xTloc = nc.dram_tensor("xTloc", [F_IN, nloc], bf16, kind="ExternalInput")
    d_w1 = nc.dram_tensor("w1", [F_IN, 512], bf16, kind="ExternalInput")
    d_a1 = nc.dram_tensor("a1", [F_IN, 8], bf16, kind="ExternalInput")
    d_w2 = nc.dram_tensor("w2", [512, C], bf16, kind="ExternalInput")
    d_a2 = nc.dram_tensor("a2", [512, 2], bf16, kind="ExternalInput")
    d_wc = nc.dram_tensor("wc", [C, OUT], f32, kind="ExternalInput")
    d_bcb = nc.dram_tensor("bcb", [G, OUT], f32, kind="ExternalInput")
    d_b1b = nc.dram_tensor("b1b", [P, 512], f32, kind="ExternalInput")
    d_b2b = nc.dram_tensor("b2b", [P, C], f32, kind="ExternalInput")
    d_srci = nc.dram_tensor("srcidx", [P, ncols], i16, kind="ExternalInput")
    d_seg = nc.dram_tensor("seg", [P, ttot * P], bf16, kind="ExternalInput")
    d_segT = nc.dram_tensor("segT", [P, ttot * P], bf16, kind="ExternalInput")
    d_poolm = nc.dram_tensor("poolm", [P, nblk * G], f32, kind="ExternalInput")
    d_out = nc.dram_tensor("out", [G, OUT], f32, kind="ExternalOutput")

    with tile.TileContext(nc) as tc:
        with tc.tile_pool(name="dram", bufs=1, space="DRAM") as dram, \
             tc.tile_pool(name="const", bufs=1) as const, \
             tc.tile_pool(name="persist", bufs=1) as persist, \
             tc.tile_pool(name="psU", bufs=2, space="PSUM") as psU, \
             tc.tile_pool(name="psD", bufs=2, space="PSUM") as psD, \
             tc.tile_pool(name="psT", bufs=2, space="PSUM") as psT, \
             tc.tile_pool(name="psA", bufs=1, space="PSUM") as psA, \
             tc.tile_pool(name="psP", bufs=1, space="PSUM") as psP, \
             tc.tile_pool(name="work", bufs=2) as work, \
             tc.tile_pool(name="rows", bufs=3) as rows:

            # ---- DRAM internals ----
            h_shard = dram.tile([nloc, ROW1], bf16)      # local D1 rows
            h_pack = dram.tile([N, ROW1], bf16, addr_space="Shared")
            h2_in = dram.tile([nloc, ROW2], bf16)        # AllGather send shard
            h2_full = dram.tile([N, ROW2], bf16, addr_space="Shared")
            pool_in = dram.tile([P, G], f32)
            pool_out = dram.tile([P, G], f32)

            # ---- SBUF constants needed by D1 (loaded first) ----
            w1_sb = const.tile([P, KC1, 512], bf16)
            nc.sync.dma_start(out=w1_sb[:], in_=d_w1.ap().rearrange("(kc p) n -> p kc n", p=P))
            a1_sb = const.tile([P, KC1, 8], bf16)
            nc.sync.dma_start(out=a1_sb[:], in_=d_a1.ap().rearrange("(kc p) n -> p kc n", p=P))

            h1T_sb = persist.tile([P, KC2, nblk * P], bf16)  # relu(out1) transposed
            pald_all = persist.tile([P, sum(tb), 4], f32)  # per-edge dst logits L1
            pald2_all = persist.tile([P, sum(tb), 1], f32)
            ald_sb = persist.tile([P, nblk, 4], bf16)     # local dst logits L1
            ald2_sb = persist.tile([P, nblk, 1], bf16)    # local dst logits L2
            # rows past nd of the last block feed matmuls (x0) - keep finite
            nc.gpsimd.memset(ald_sb[:], 0.0)
            nc.gpsimd.memset(ald2_sb[:], 0.0)

            do_e1 = phases in ('d1e1', 'd1e1d2', 'nocoll', 'full')
            do_d2 = phases in ('d1e1d2', 'nocoll', 'full')
            do_e2 = phases in ('nocoll', 'full')
            do_coll = phases == 'full'

            # ====== D1: h1 = x_shard @ W1 (local nodes only, sharded) =======
            with tc.tile_pool(name="xpool", bufs=1) as xpool:
                xg = xpool.tile([P, KC1, nloc], bf16, name="xg")
                nc.sync.dma_start(
                    out=xg[:], in_=d_xTloc.ap().rearrange("(kc p) n -> p kc n", p=P))
                for j in range(nblk):
                    nd = min(P, nloc - j * P)
                    ph = psU.tile([P, 512], f32, tag="U")
                    pal = psD.tile([P, 8], f32, tag="den")
                    for kc in range(KC1):
                        lhs = xg[:, kc, j * P: j * P + nd]
                        nc.tensor.matmul(out=ph[0:nd, :], lhsT=lhs,
                                         rhs=w1_sb[:, kc, :],
                                         start=(kc == 0), stop=(kc == KC1 - 1))
                        nc.tensor.matmul(out=pal[0:nd, :], lhsT=lhs,
                                         rhs=a1_sb[:, kc, :],
                                         start=(kc == 0), stop=(kc == KC1 - 1))
                    hrow = rows.tile([P, ROW1], bf16, tag="hrow", bufs=2)
                    nc.gpsimd.memset(hrow[0:nd, 516:ROW1], 0.0)
                    nc.vector.tensor_copy(out=hrow[0:nd, 0:512], in_=ph[0:nd, :])
                    nc.vector.tensor_copy(out=hrow[0:nd, 512:516], in_=pal[0:nd, 0:4])
                    nc.sync.dma_start(out=h_shard[j * P: j * P + nd, :],
                                      in_=hrow[0:nd, :])
                    nc.vector.tensor_copy(out=ald_sb[0:nd, j, :], in_=pal[0:nd, 4:8])

            # ---- exchange layer-1 gather table ----
            if do_coll:
                nc.gpsimd.collective_compute(
                    "AllGather", mybir.AluOpType.bypass,
                    replica_groups=[list(range(n_cores))],
                    ins=[h_shard.opt()], outs=[h_pack.opt()])
            else:
                nc.sync.dma_start(out=h_pack[0:nloc, :], in_=h_shard[:, :])


            # tile index -> owning block
            tile_blk = []
            for _b in range(nblk):
                tile_blk += [_b] * tb[_b]

            # ---- pald (L1 per-edge dst logits) streamed during AllGather ----
            CH = 32
            with tc.tile_pool(name="segs", bufs=2) as segs:
                for g0 in range(0, ttot, CH):
                    nt = min(CH, ttot - g0)
                    sT = segs.tile([P, CH, P], bf16, tag="sT")
                    nc.sync.dma_start(
                        out=sT[:, 0:nt, :],
                        in_=d_segT.ap().rearrange("p (t q) -> p t q", q=P)[:, g0:g0 + nt, :])
                    pch = psA.tile([P, CH, 4], f32, tag="pald")
                    for k in range(nt):
                        nc.tensor.matmul(out=pch[:, k, :], lhsT=sT[:, k, :],
                                         rhs=ald_sb[:, tile_blk[g0 + k], :],
                                         start=True, stop=True)
                    nc.vector.tensor_copy(out=pald_all[:, g0:g0 + nt, :],
                                          in_=pch[:, 0:nt, :])

            # ---- remaining SBUF constants (needed from E1 on) ----
            w2_sb = const.tile([P, KC2, C], bf16)
            nc.sync.dma_start(out=w2_sb[:], in_=d_w2.ap().rearrange("(kc p) n -> p kc n", p=P))
            a2_sb = const.tile([P, KC2, 2], bf16)
            nc.sync.dma_start(out=a2_sb[:], in_=d_a2.ap().rearrange("(kc p) n -> p kc n", p=P))
            wc_sb = const.tile([P, OUT], f32)
            nc.sync.dma_start(out=wc_sb[:], in_=d_wc[:, :])
            bcb_sb = const.tile([G, OUT], f32)
            nc.sync.dma_start(out=bcb_sb[:], in_=d_bcb[:, :])
            b1b_sb = const.tile([P, 512], f32)
            nc.sync.dma_start(out=b1b_sb[:], in_=d_b1b[:, :])
            b2b_sb = const.tile([P, C], f32)
            nc.sync.dma_start(out=b2b_sb[:], in_=d_b2b[:, :])
            srci_sb = const.tile([P, ncols], i16)
            nc.sync.dma_start(out=srci_sb[:], in_=d_srci[:, :])
            poolm_sb = const.tile([P, nblk, G], f32)
            nc.sync.dma_start(out=poolm_sb[:], in_=d_poolm.ap())
            ident = const.tile([P, P], bf16)
            make_identity(nc, ident[:])

            def emit_d2(j):
                nd2 = min(P, nloc - j * P)
                o1b_j, _ = o1bs[j]
                for kc in range(KC2):
                    tp = psT.tile([P, P], bf16, tag="tp")
                    nc.tensor.transpose(out=tp[:, 0:nd2],
                                        in_=o1b_j[0:nd2, kc * P:(kc + 1) * P],
                                        identity=ident[0:nd2, 0:nd2])
                    nc.scalar.activation(h1T_sb[:, kc, j * P: j * P + nd2],
                                         tp[:, 0:nd2],
                                         mybir.ActivationFunctionType.Copy)
                p2 = psU.tile([P, C], f32, tag="U")
                p2a = psD.tile([P, 2], f32, tag="den")
                for kc in range(KC2):
                    lhs = h1T_sb[:, kc, j * P: j * P + nd2]
                    nc.tensor.matmul(out=p2[0:nd2, :], lhsT=lhs, rhs=w2_sb[:, kc, :],
                                     start=(kc == 0), stop=(kc == KC2 - 1))
                    nc.tensor.matmul(out=p2a[0:nd2, :], lhsT=lhs, rhs=a2_sb[:, kc, :],
                                     start=(kc == 0), stop=(kc == KC2 - 1))
                r2 = rows.tile([P, ROW2], bf16, tag="r2", name="r2")
                nc.gpsimd.memset(r2[0:nd2, C + 1:ROW2], 0.0)
                nc.scalar.activation(r2[0:nd2, 0:C], p2[0:nd2, :],
                                     mybir.ActivationFunctionType.Copy)
                nc.scalar.activation(r2[0:nd2, C:C + 1], p2a[0:nd2, 0:1],
                                     mybir.ActivationFunctionType.Copy)
                nc.sync.dma_start(out=h2_in[j * P: j * P + nd2, :], in_=r2[0:nd2, :])
                nc.scalar.activation(ald2_sb[0:nd2, j, :], p2a[0:nd2, 1:2],
                                     mybir.ActivationFunctionType.Copy)

            o1bs = {}
            # ========== E1 (+ D2 of block b-1, whose deps are ready) ========
            for b in range(nblk if do_e1 else 0):
                nd = min(P, nloc - b * P)
                t_b = tb[b]
                to = toffs[b]
                nidx = t_b * P
                cb = to * 8
                hg = work.tile([P, t_max, ROW1], bf16, tag="hg", bufs=5)
                nc.gpsimd.dma_gather(hg[:, 0:t_b, :], h_pack[:, :],
                                     srci_sb[:, cb:cb + t_b * 8],
                                     nidx, nidx, ROW1, single_packet=False)
                segc = work.tile([P, t_max, P], bf16, tag="segc", bufs=3)
                nc.sync.dma_start(
                    out=segc[:, 0:t_b, :],
                    in_=d_seg.ap().rearrange("p (t q) -> p t q", q=P)[:, to:to + t_b, :])

                if do_d2 and b > 0:
                    emit_d2(b - 1)

                # self-loop row block: local shard rows (contiguous)
                hself = work.tile([P, 516], bf16, tag="hself")
                nc.sync.dma_start(out=hself[0:nd, :],
                                  in_=h_shard[b * P: b * P + nd, 0:516])

                s32 = work.tile([P, t_max, 4], f32, tag="s32")
                nc.vector.tensor_tensor(out=s32[:, 0:t_b, :],
                                        in0=hg[:, 0:t_b, 512:516],
                                        in1=pald_all[:, to:to + t_b, :],
                                        op=mybir.AluOpType.add)
                ea = work.tile([P, t_max, 4], f32, tag="ea")
                nc.scalar.activation(ea[:, 0:t_b, :], s32[:, 0:t_b, :],
                                     mybir.ActivationFunctionType.Exp)
                eb = work.tile([P, t_max, 4], f32, tag="eb")
                nc.scalar.activation(eb[:, 0:t_b, :], s32[:, 0:t_b, :],
                                     mybir.ActivationFunctionType.Exp,
                                     scale=NEG_SLOPE)
                pbf = work.tile([P, t_max, 4], bf16, tag="pbf")
                nc.vector.tensor_tensor(out=pbf[:, 0:t_b, :], in0=ea[:, 0:t_b, :],
                                        in1=eb[:, 0:t_b, :], op=mybir.AluOpType.max)

                # self-loop attention: p_self = exp(leaky(als + ald))
                sself = work.tile([P, 4], f32, tag="sself")
                nc.vector.tensor_tensor(out=sself[0:nd, :], in0=hself[0:nd, 512:516],
                                        in1=ald_sb[0:nd, b, :], op=mybir.AluOpType.add)
                eaself = work.tile([P, 4], f32, tag="eaself")
                nc.scalar.activation(eaself[0:nd, :], sself[0:nd, :],
                                     mybir.ActivationFunctionType.Exp)
                ebself = work.tile([P, 4], f32, tag="ebself")
                nc.scalar.activation(ebself[0:nd, :], sself[0:nd, :],
                                     mybir.ActivationFunctionType.Exp,
                                     scale=NEG_SLOPE)
                pself = work.tile([P, 4], f32, tag="pself")
                nc.vector.tensor_tensor(out=pself[0:nd, :], in0=eaself[0:nd, :],
                                        in1=ebself[0:nd, :], op=mybir.AluOpType.max)

                th = t_b // 2
                hgA = hg[:, 0:th, 0:512].rearrange("p t (h c) -> p t h c", h=H1)
                nc.vector.tensor_tensor(
                    out=hgA, in0=hgA,
                    in1=pbf[:, 0:th, :].unsqueeze(3).to_broadcast([P, th, H1, C]),
                    op=mybir.AluOpType.mult)
                hgB = hg[:, th:t_b, 0:512].rearrange("p t (h c) -> p t h c", h=H1)
                nc.vector.tensor_tensor(
                    out=hgB, in0=hgB,
                    in1=pbf[:, th:t_b, :].unsqueeze(3).to_broadcast([P, t_b - th, H1, C]),
                    op=mybir.AluOpType.mult)

                den = psD.tile([P, 4], f32, tag="den")
                for t in range(t_b):
                    nc.tensor.matmul(out=den[:, :], lhsT=segc[:, t, :],
                                     rhs=pbf[:, t, :],
                                     start=(t == 0), stop=(t == t_b - 1))
                U = psU.tile([P, 512], f32, tag="U")
                for t in range(t_b):
                    nc.tensor.matmul(out=U[:, :], lhsT=segc[:, t, :],
                                     rhs=hg[:, t, 0:512],
                                     start=(t == 0), stop=(t == t_b - 1))

                # add self-loop contribution, normalize, bias, relu
                o1 = work.tile([P, 512], f32, tag="o1")
                nc.vector.tensor_tensor(
                    out=o1[0:nd, :].rearrange("p (h c) -> p h c", h=H1),
                    in0=hself[0:nd, 0:512].rearrange("p (h c) -> p h c", h=H1),
                    in1=pself[0:nd, :].unsqueeze(2).to_broadcast([nd, H1, C]),
                    op=mybir.AluOpType.mult)
                dtot = work.tile([P, 4], f32, tag="dtot")
                nc.vector.tensor_tensor(out=dtot[0:nd, :], in0=den[0:nd, :],
                                        in1=pself[0:nd, :], op=mybir.AluOpType.add)
                rec = work.tile([P, 4], f32, tag="rec")
                nc.vector.reciprocal(rec[0:nd, :], dtot[0:nd, :])
                nc.vector.tensor_tensor(out=o1[0:nd, :], in0=U[0:nd, :],
                                        in1=o1[0:nd, :], op=mybir.AluOpType.add)
                o1h = o1[0:nd, :].rearrange("p (h c) -> p h c", h=H1)
                nc.vector.tensor_tensor(
                    out=o1h, in0=o1h,
                    in1=rec[0:nd, :].unsqueeze(2).to_broadcast([nd, H1, C]),
                    op=mybir.AluOpType.mult)
                nc.vector.tensor_tensor(out=o1[0:nd, :], in0=o1[0:nd, :],
                                        in1=b1b_sb[0:nd, :], op=mybir.AluOpType.add)
                o1b = work.tile([P, 512], bf16, tag="o1b", bufs=2)
                nc.scalar.activation(o1b[0:nd, :], o1[0:nd, :],
                                     mybir.ActivationFunctionType.Relu)
                o1bs[b] = (o1b, nd)

            # ---- D2 for the final block ----
            if do_d2:
                emit_d2(nblk - 1)

            # ---- exchange layer-2 inputs ----
            if do_coll:
                nc.gpsimd.collective_compute(
                    "AllGather", mybir.AluOpType.bypass,
                    replica_groups=[list(range(n_cores))],
                    ins=[h2_in.opt()], outs=[h2_full.opt()])
            elif do_e2:
                nc.sync.dma_start(out=h2_full[0:nloc, :], in_=h2_in[:, :])

            # ---- pald2 streamed during the second AllGather ----
            with tc.tile_pool(name="segs2", bufs=2) as segs2:
                for g0 in range(0, ttot, CH):
                    nt = min(CH, ttot - g0)
                    sT2 = segs2.tile([P, CH, P], bf16, tag="sT2")
                    nc.sync.dma_start(
                        out=sT2[:, 0:nt, :],
                        in_=d_segT.ap().rearrange("p (t q) -> p t q", q=P)[:, g0:g0 + nt, :])
                    pch2 = psA.tile([P, CH, 4], f32, tag="pald")
                    for k in range(nt):
                        nc.tensor.matmul(out=pch2[:, k, 0:1], lhsT=sT2[:, k, :],
                                         rhs=ald2_sb[:, tile_blk[g0 + k], :],
                                         start=True, stop=True)
                    nc.vector.tensor_copy(out=pald2_all[:, g0:g0 + nt, :],
                                          in_=pch2[:, 0:nt, 0:1])

            # ================= E2: layer-2 edge phase =======================
            poolT = psP.tile([P, G], f32, tag="poolT")
            for b in range(nblk if do_e2 else 0):
                nd = min(P, nloc - b * P)
                t_b = tb[b]
                to = toffs[b]
                nidx = t_b * P
                cb = to * 8
                hg2 = work.tile([P, t_max, ROW2], bf16, tag="hg2", bufs=3)
                nc.gpsimd.dma_gather(hg2[:, 0:t_b, :], h2_full[:, :],
                                     srci_sb[:, cb:cb + t_b * 8],
                                     nidx, nidx, ROW2, single_packet=False)
                segc2 = work.tile([P, t_max, P], bf16, tag="segc", bufs=3)
                nc.sync.dma_start(
                    out=segc2[:, 0:t_b, :],
                    in_=d_seg.ap().rearrange("p (t q) -> p t q", q=P)[:, to:to + t_b, :])

                h2self = work.tile([P, C + 1], bf16, tag="h2self")
                nc.sync.dma_start(out=h2self[0:nd, :],
                                  in_=h2_in[b * P: b * P + nd, 0:C + 1])

                s2 = work.tile([P, t_max, 1], f32, tag="s2")
                nc.vector.tensor_tensor(out=s2[:, 0:t_b, :], in0=hg2[:, 0:t_b, C:C + 1],
                                        in1=pald2_all[:, to:to + t_b, :],
                                        op=mybir.AluOpType.add)
                ea2 = work.tile([P, t_max, 1], f32, tag="ea2")
                nc.scalar.activation(ea2[:, 0:t_b, :], s2[:, 0:t_b, :],
                                     mybir.ActivationFunctionType.Exp)
                eb2 = work.tile([P, t_max, 1], f32, tag="eb2")
                nc.scalar.activation(eb2[:, 0:t_b, :], s2[:, 0:t_b, :],
                                     mybir.ActivationFunctionType.Exp,
                                     scale=NEG_SLOPE)
                pbf2 = work.tile([P, t_max, 1], bf16, tag="pbf2")
                nc.vector.tensor_tensor(out=pbf2[:, 0:t_b, :], in0=ea2[:, 0:t_b, :],
                                        in1=eb2[:, 0:t_b, :], op=mybir.AluOpType.max)

                s2self = work.tile([P, 1], f32, tag="s2self")
                nc.vector.tensor_tensor(out=s2self[0:nd, :], in0=h2self[0:nd, C:C + 1],
                                        in1=ald2_sb[0:nd, b, :], op=mybir.AluOpType.add)
                ea2self = work.tile([P, 1], f32, tag="ea2self")
                nc.scalar.activation(ea2self[0:nd, :], s2self[0:nd, :],
                                     mybir.ActivationFunctionType.Exp)
                eb2self = work.tile([P, 1], f32, tag="eb2self")
                nc.scalar.activation(eb2self[0:nd, :], s2self[0:nd, :],
                                     mybir.ActivationFunctionType.Exp,
                                     scale=NEG_SLOPE)
                p2self = work.tile([P, 1], f32, tag="p2self")
                nc.vector.tensor_tensor(out=p2self[0:nd, :], in0=ea2self[0:nd, :],
                                        in1=eb2self[0:nd, :], op=mybir.AluOpType.max)

                nc.vector.tensor_tensor(
                    out=hg2[:, 0:t_b, 0:C], in0=hg2[:, 0:t_b, 0:C],
                    in1=pbf2[:, 0:t_b, 0:1].to_broadcast([P, t_b, C]),
                    op=mybir.AluOpType.mult)

                den2 = psD.tile([P, 1], f32, tag="den")
                for t in range(t_b):
                    nc.tensor.matmul(out=den2[:, :], lhsT=segc2[:, t, :],
                                     rhs=pbf2[:, t, :],
                                     start=(t == 0), stop=(t == t_b - 1))
                U2 = psU.tile([P, C], f32, tag="U")
                for t in range(t_b):
                    nc.tensor.matmul(out=U2[:, :], lhsT=segc2[:, t, :],
                                     rhs=hg2[:, t, 0:C],
                                     start=(t == 0), stop=(t == t_b - 1))

                o2self = work.tile([P, C], f32, tag="o2self")
                nc.vector.tensor_tensor(out=o2self[0:nd, :], in0=h2self[0:nd, 0:C],
                                        in1=p2self[0:nd, 0:1].to_broadcast([nd, C]),
                                        op=mybir.AluOpType.mult)
                d2tot = work.tile([P, 1], f32, tag="d2tot")
                nc.vector.tensor_tensor(out=d2tot[0:nd, :], in0=den2[0:nd, :],
                                        in1=p2self[0:nd, :], op=mybir.AluOpType.add)
                rec2 = work.tile([P, 1], f32, tag="rec2")
                nc.vector.reciprocal(rec2[0:nd, :], d2tot[0:nd, :])
                o2 = work.tile([P, C], f32, tag="o2")
                nc.vector.tensor_tensor(out=o2[0:nd, :], in0=U2[0:nd, :],
                                        in1=o2self[0:nd, :], op=mybir.AluOpType.add)
                nc.vector.tensor_tensor(out=o2[0:nd, :], in0=o2[0:nd, :],
                                        in1=rec2[0:nd, 0:1].to_broadcast([nd, C]),
                                        op=mybir.AluOpType.mult)
                nc.vector.tensor_tensor(out=o2[0:nd, :], in0=o2[0:nd, :],
                                        in1=b2b_sb[0:nd, :], op=mybir.AluOpType.add)
                o2r = work.tile([P, C], f32, tag="o2r")
                nc.scalar.activation(o2r[0:nd, :], o2[0:nd, :],
                                     mybir.ActivationFunctionType.Relu)
                nc.tensor.matmul(out=poolT[:, :], lhsT=o2r[0:nd, :],
                                 rhs=poolm_sb[0:nd, b, :],
                                 start=(b == 0), stop=(b == nblk - 1))

            # ================= tail: pool exchange + classifier =============
            if not do_e2:
                dummy = work.tile([G, OUT], f32, tag="dummy")
                nc.vector.tensor_copy(out=dummy[:], in_=bcb_sb[:])
                nc.sync.dma_start(out=d_out[:, :], in_=dummy[:])
            else:
              poolT_sb = work.tile([P, G], f32, tag="poolT_sb")
              nc.scalar.activation(poolT_sb[:], poolT[:, :],
                                   mybir.ActivationFunctionType.Copy)
              nc.sync.dma_start(out=pool_in[:, :], in_=poolT_sb[:])
              if do_coll:
                  nc.gpsimd.collective_compute(
                      "AllReduce", mybir.AluOpType.add,
                      replica_groups=[list(range(n_cores))],
                      ins=[pool_in.opt()], outs=[pool_out.opt()])
              else:
                  nc.sync.dma_start(out=pool_out[:, :], in_=pool_in[:, :])
              poolF_sb = work.tile([P, G], f32, tag="poolF_sb")
              nc.sync.dma_start(out=poolF_sb[:], in_=pool_out[:, :])
              ofin = psD.tile([G, OUT], f32, tag="den")
              nc.tensor.matmul(out=ofin[:, :], lhsT=poolF_sb[:], rhs=wc_sb[:],
                               start=True, stop=True)
              ofin_sb = work.tile([G, OUT], f32, tag="ofin_sb")
              nc.vector.tensor_tensor(out=ofin_sb[:], in0=ofin[:, :], in1=bcb_sb[:],
                                      op=mybir.AluOpType.add)
              nc.sync.dma_start(out=d_out[:, :], in_=ofin_sb[:])

    nc.compile()
    return nc


# ------------------------------------------------------------------
#  runner
# ------------------------------------------------------------------

_CACHE = {}


def _get_nc(meta):
    key = (meta['n_cores'], meta['nblk'], meta['tb'], meta['nloc'])
    if key not in _CACHE:
        _CACHE[key] = _build(meta)
    return _CACHE[key]


def _in_maps(common, per_core):
    maps = []
    for pc in per_core:
        m = dict(common)
        m.update(pc)
        maps.append(m)
    return maps


def kernel(**inputs) -> np.ndarray:
    common, per_core, meta = _prep(**inputs)
    nc = _get_nc(meta)
    from concourse.bass_utils import run_bass_kernel_spmd
    res = run_bass_kernel_spmd(nc, _in_maps(common, per_core),
                               core_ids=list(range(meta['n_cores'])))
    return np.asarray(res.results[0]['out'], np.float32).reshape(-1)


# revision 30
# speedup vs baseline: 1.0514x; 1.0385x over previous
"""Trainium2 Bass kernel for a 2-layer GAT + global mean pool + linear head.

Strategy (8 NeuronCores, SPMD single program, per-core data):
  - Nodes are partitioned into 8 contiguous shards of 1250; each core owns the
    edges whose dst falls in its shard (grouped by dst block of 128, sorted).
  - D1 is SHARDED: each core computes h1 = x_shard @ W1 for its 1250 nodes
    (als/ald logits folded into the matmul via host precompute), writes a
    node-major row table [nloc, 640] bf16 (cols 0:512 = h1, 512:516 = als),
    then an 8-rank AllGather (Shared scratchpad output) builds the full
    gather table [N, 640].  Dst logits (ald) stay in SBUF.
  - Edge phase: per dst-block, NON-self-loop edge source rows are fetched
    with dma_gather (SWDGE; cost ~8ns/idx, so self-loops - whose rows are
    the block's own contiguous shard rows - are handled by a direct DMA +
    vector path instead, and per-block tile counts are trimmed to the max
    actual edge count across cores).  Per-edge dst logits come from the
    tensor engine as segT_tile^T @ ald_block.  Segment-softmax runs without
    max-subtraction; normalization after aggregation via seg matmuls in PSUM.
  - D2 is interleaved into the E1 block loop; layer 2 exchange via a second
    AllGather; pooled partials with an AllReduce.
"""
import os
import sys
import numpy as np

for _p in ("/opt/trn_rl_repo", "/root/.axon_site/_ro/trn_rl_repo"):
    if os.path.isdir(_p) and _p not in sys.path:
        sys.path.append(_p)

import ml_dtypes

BF16 = ml_dtypes.bfloat16

# -------- problem constants (hardcoded per contest rules) --------
N = 10000
E = 160000
F_IN = 768
H1 = 4
C = 128
OUT = 10
G = 16
NEG_SLOPE = 0.2
P = 128
N_CORES = 8
KC1 = F_IN // P          # 6 k-chunks for layer-1 matmul
KC2 = (H1 * C) // P      # 4 k-chunks for layer-2 matmul
ROW1 = 640               # h-pack row: 512 h | 4 als | 124 pad   (1280 B bf16)
ROW2 = 256               # h2-pack row: 128 h2 | 1 als2 | 127 pad (512 B)


def _bf(x):
    return np.ascontiguousarray(np.asarray(x, dtype=np.float32).astype(BF16))


def _prep(x, edge_index, batch, W1, att_src1, att_dst1, b1, W2, att_src2,
          att_dst2, b2, Wc, bc, n_cores=N_CORES):
    """Host-side index/layout preprocessing. Returns (common, per_core, meta)."""
    x = np.asarray(x, np.float32)
    edge_index = np.asarray(edge_index, np.int64)
    batch = np.asarray(batch, np.int64)
    nloc = N // n_cores
    nblk = (nloc + P - 1) // P
    # self-loops (one per node, appended by the reference) are handled by a
    # dense local path on device; only the raw edges go through the gather.
    src = edge_index[0]
    dst = edge_index[1]

    W1 = np.asarray(W1, np.float32)
    W2 = np.asarray(W2, np.float32)
    W1r = W1.reshape(F_IN, H1, C)
    A1 = np.concatenate([
        np.einsum('khc,hc->kh', W1r, np.asarray(att_src1, np.float32)),
        np.einsum('khc,hc->kh', W1r, np.asarray(att_dst1, np.float32)),
    ], axis=1)                                  # [768, 8]
    A2 = np.stack([W2 @ np.asarray(att_src2, np.float32)[0],
                   W2 @ np.asarray(att_dst2, np.float32)[0]], axis=1)  # [512, 2]

    cnt = np.bincount(batch, minlength=G).astype(np.float32)
    inv_cnt = 1.0 / np.maximum(cnt, 1.0)

    # per-core edge grouping by dst block
    core_blocks = []
    blk_cnt = np.zeros((n_cores, nblk), dtype=np.int64)
    for c in range(n_cores):
        lo = c * nloc
        m = (dst >= lo) & (dst < lo + nloc)
        s_c, d_c = src[m], dst[m] - lo
        order = np.argsort(d_c, kind='stable')
        s_c, d_c = s_c[order], d_c[order]
        blocks = []
        for b in range(nblk):
            bm = (d_c >= b * P) & (d_c < min((b + 1) * P, nloc))
            blocks.append((s_c[bm], d_c[bm] - b * P))
            blk_cnt[c, b] = bm.sum()
        core_blocks.append(blocks)
    # per-block tile count: max edges across cores, ceil-div 128
    tb = tuple(int(v) for v in
               (blk_cnt.max(axis=0) + P - 1) // P)
    ttot = int(sum(tb))

    def idx_wrap(vals):
        v = np.asarray(vals, dtype=np.int16)
        out = np.zeros((16, len(v) // 16), dtype=np.int16)
        i = np.arange(len(v))
        out[i % 16, i // 16] = v
        return np.tile(out, (8, 1))

    common = dict(
        w1=_bf(W1), a1=_bf(A1), w2=_bf(W2), a2=_bf(A2),
        wc=np.ascontiguousarray(np.asarray(Wc, np.float32)),
        bcb=np.ascontiguousarray(np.tile(np.asarray(bc, np.float32), (G, 1))),
        b1b=np.ascontiguousarray(np.tile(np.asarray(b1, np.float32), (P, 1))),
        b2b=np.ascontiguousarray(np.tile(np.asarray(b2, np.float32), (P, 1))),
    )

    per_core = []
    for c in range(n_cores):
        lo = c * nloc
        srccols = []
        seg = np.zeros((ttot, P, P), dtype=BF16)
        toff = 0
        for b in range(nblk):
            sb, db = core_blocks[c][b]
            ne = tb[b] * P
            s_pad = np.zeros(ne, dtype=np.int64); s_pad[:len(sb)] = sb
            srccols.append(idx_wrap(s_pad))
            ei = np.arange(len(db))
            seg[toff + ei // P, ei % P, db] = BF16(1.0)
            toff += tb[b]
        poolm = np.zeros((nblk * P, G), dtype=np.float32)
        gg = batch[lo:lo + nloc]
        poolm[np.arange(nloc), gg] = inv_cnt[gg]
        per_core.append(dict(
            xTloc=_bf(x[lo:lo + nloc].T),       # [768, nloc]
            srcidx=np.ascontiguousarray(np.concatenate(srccols, axis=1)),
            # seg shipped pre-transposed: [128 (edge), ttot * 128 (tile, dst)]
            seg=np.ascontiguousarray(seg.transpose(1, 0, 2).reshape(P, ttot * P)),
            # segT: [128 (dst), ttot * 128 (tile, edge)]
            segT=np.ascontiguousarray(seg.transpose(2, 0, 1).reshape(P, ttot * P)),
            # poolm shipped pre-transposed: [128 (node-in-chunk), nblk * 16]
            poolm=np.ascontiguousarray(
                poolm.reshape(nblk, P, G).transpose(1, 0, 2).reshape(P, nblk * G)),
        ))
    meta = dict(n_cores=n_cores, nloc=nloc, nblk=nblk, tb=tb)
    return common, per_core, meta


# ------------------------------------------------------------------
#  device program
# ------------------------------------------------------------------

def _build(meta, phases='full'):
    from concourse import bass, bacc, tile, mybir
    from concourse.masks import make_identity

    n_cores, nloc, nblk, tb = (meta['n_cores'], meta['nloc'],
                               meta['nblk'], meta['tb'])
    ttot = sum(tb)
    t_max = max(tb)
    toffs = [sum(tb[:b]) for b in range(nblk)]
    ncols = ttot * 8                      # idx tensor cols
    bf16, f32, i16 = mybir.dt.bfloat16, mybir.dt.float32, mybir.dt.int16

    nc = bacc.Bacc("TRN2", target_bir_lowering=False, debug=False,
                   num_devices=n_cores)

    # ---- I/O ----
    d_xTloc = nc.dram_tensor("xTloc", [F_IN, nloc], bf16, kind="ExternalInput")
    d_w1 = nc.dram_tensor("w1", [F_IN, 512], bf16, kind="ExternalInput")
    d_a1 = nc.dram_tensor("a1", [F_IN, 8], bf16, kind="ExternalInput")
    d_w2 = nc.dram_tensor("w2", [512, C], bf16, kind="ExternalInput")
    d_a2 = nc.dram_tensor("a2", [512, 2], bf16, kind="ExternalInput")
    d_wc = nc.dram_tensor("wc", [C, OUT], f32, kind="ExternalInput")
    d_bcb = nc.dram_tensor("bcb", [G, OUT], f32, kind="ExternalInput")
    d_b1b = nc.dram_tensor("b1b", [P, 512], f32, kind="ExternalInput")
    d_b2b = nc.dram_tensor("b2b", [P, C], f32, kind="ExternalInput")
    d_srci = nc.dram_tensor("srcidx", [P, ncols], i16, kind="ExternalInput")
    d_seg = nc.dram_tensor("seg", [P, ttot * P], bf16, kind="ExternalInput")
    d_segT = nc.dram_tensor("segT", [P, ttot * P], bf16, kind="ExternalInput")
    d_poolm = nc.dram_tensor("poolm", [P, nblk * G], f32, kind="ExternalInput")
    d_out = nc.dram_tensor("out", [G, OUT], f32, kind="ExternalOutput")

    with tile.TileContext(nc) as tc:
        with tc.tile_pool(name="dram", bufs=1, space="DRAM") as dram, \
             tc.tile_pool(name="const", bufs=1) as const, \
             tc.tile_pool(name="persist", bufs=1) as persist, \
             tc.tile_pool(name="psU", bufs=2, space="PSUM") as psU, \
             tc.tile_pool(name="psD", bufs=2, space="PSUM") as psD, \
             tc.tile_pool(name="psT", bufs=2, space="PSUM") as psT, \
             tc.tile_pool(name="psA", bufs=1, space="PSUM") as psA, \
             tc.tile_pool(name="psP", bufs=1, space="PSUM") as psP, \
             tc.tile_pool(name="work", bufs=2) as work, \
             tc.tile_pool(name="rows", bufs=3) as rows:

            # ---- DRAM internals ----
            h_shard = dram.tile([nloc, ROW1], bf16)      # local D1 rows
            h_pack = dram.tile([N, ROW1], bf16, addr_space="Shared")
            h2_in = dram.tile([nloc, ROW2], bf16)        # AllGather send shard
            h2_full = dram.tile([N, ROW2], bf16, addr_space="Shared")
            pool_in = dram.tile([P, G], f32)
            pool_out = dram.tile([P, G], f32)

            # ---- SBUF constants needed by D1 (loaded first) ----
            w1_sb = const.tile([P, KC1, 512], bf16)
            nc.sync.dma_start(out=w1_sb[:], in_=d_w1.ap().rearrange("(kc p) n -> p kc n", p=P))
            a1_sb = const.tile([P, KC1, 8], bf16)
            nc.sync.dma_start(out=a1_sb[:], in_=d_a1.ap().rearrange("(kc p) n -> p kc n", p=P))

            h1T_sb = persist.tile([P, KC2, nblk * P], bf16)  # relu(out1) transposed
            pald_all = persist.tile([P, sum(tb), 4], f32)  # per-edge dst logits L1
            pald2_all = persist.tile([P, sum(tb), 1], f32)
            pself_all = persist.tile([P, nblk, 4], f32)   # self-loop attn L1
            p2self_all = persist.tile([P, nblk, 1], f32)  # self-loop attn L2
            ald_sb = persist.tile([P, nblk, 4], bf16)     # local dst logits L1
            ald2_sb = persist.tile([P, nblk, 1], bf16)    # local dst logits L2
            # rows past nd of the last block feed matmuls (x0) - keep finite
            nc.gpsimd.memset(ald_sb[:], 0.0)
            nc.gpsimd.memset(ald2_sb[:], 0.0)

            do_e1 = phases in ('d1e1', 'd1e1d2', 'nocoll', 'full')
            do_d2 = phases in ('d1e1d2', 'nocoll', 'full')
            do_e2 = phases in ('nocoll', 'full')
            do_coll = phases == 'full'

            # ====== D1: h1 = x_shard @ W1 (local nodes only, sharded) =======
            with tc.tile_pool(name="xpool", bufs=1) as xpool:
                xg = xpool.tile([P, KC1, nloc], bf16, name="xg")
                nc.sync.dma_start(
                    out=xg[:], in_=d_xTloc.ap().rearrange("(kc p) n -> p kc n", p=P))
                for j in range(nblk):
                    nd = min(P, nloc - j * P)
                    ph = psU.tile([P, 512], f32, tag="U")
                    pal = psD.tile([P, 8], f32, tag="den")
                    for kc in range(KC1):
                        lhs = xg[:, kc, j * P: j * P + nd]
                        nc.tensor.matmul(out=ph[0:nd, :], lhsT=lhs,
                                         rhs=w1_sb[:, kc, :],
                                         start=(kc == 0), stop=(kc == KC1 - 1))
                        nc.tensor.matmul(out=pal[0:nd, :], lhsT=lhs,
                                         rhs=a1_sb[:, kc, :],
                                         start=(kc == 0), stop=(kc == KC1 - 1))
                    hrow = rows.tile([P, ROW1], bf16, tag="hrow", bufs=2)
                    nc.gpsimd.memset(hrow[0:nd, 516:ROW1], 0.0)
                    nc.vector.tensor_copy(out=hrow[0:nd, 0:512], in_=ph[0:nd, :])
                    nc.vector.tensor_copy(out=hrow[0:nd, 512:516], in_=pal[0:nd, 0:4])
                    nc.sync.dma_start(out=h_shard[j * P: j * P + nd, :],
                                      in_=hrow[0:nd, :])
                    nc.vector.tensor_copy(out=ald_sb[0:nd, j, :], in_=pal[0:nd, 4:8])

            # ---- exchange layer-1 gather table ----
            if do_coll:
                nc.gpsimd.collective_compute(
                    "AllGather", mybir.AluOpType.bypass,
                    replica_groups=[list(range(n_cores))],
                    ins=[h_shard.opt()], outs=[h_pack.opt()])
            else:
                nc.sync.dma_start(out=h_pack[0:nloc, :], in_=h_shard[:, :])


            # tile index -> owning block
            tile_blk = []
            for _b in range(nblk):
                tile_blk += [_b] * tb[_b]

            # ---- pald (L1 per-edge dst logits) streamed during AllGather ----
            CH = 32
            with tc.tile_pool(name="segs", bufs=2) as segs:
                for g0 in range(0, ttot, CH):
                    nt = min(CH, ttot - g0)
                    sT = segs.tile([P, CH, P], bf16, tag="sT")
                    nc.sync.dma_start(
                        out=sT[:, 0:nt, :],
                        in_=d_segT.ap().rearrange("p (t q) -> p t q", q=P)[:, g0:g0 + nt, :])
                    pch = psA.tile([P, CH, 4], f32, tag="pald")
                    for k in range(nt):
                        nc.tensor.matmul(out=pch[:, k, :], lhsT=sT[:, k, :],
                                         rhs=ald_sb[:, tile_blk[g0 + k], :],
                                         start=True, stop=True)
                    nc.vector.tensor_copy(out=pald_all[:, g0:g0 + nt, :],
                                          in_=pch[:, 0:nt, :])

            # ---- self-loop attention precompute (runs during AllGather) ----
            for b in range(nblk):
                nd = min(P, nloc - b * P)
                alsb = work.tile([P, 4], bf16, tag="alsb")
                nc.sync.dma_start(out=alsb[0:nd, :],
                                  in_=h_shard[b * P: b * P + nd, 512:516])
                sselfp = work.tile([P, 4], f32, tag="sselfp")
                nc.vector.tensor_tensor(out=sselfp[0:nd, :], in0=alsb[0:nd, :],
                                        in1=ald_sb[0:nd, b, :], op=mybir.AluOpType.add)
                easp = work.tile([P, 4], f32, tag="easp")
                nc.scalar.activation(easp[0:nd, :], sselfp[0:nd, :],
                                     mybir.ActivationFunctionType.Exp)
                ebsp = work.tile([P, 4], f32, tag="ebsp")
                nc.scalar.activation(ebsp[0:nd, :], sselfp[0:nd, :],
                                     mybir.ActivationFunctionType.Exp,
                                     scale=NEG_SLOPE)
                nc.vector.tensor_tensor(out=pself_all[0:nd, b, :], in0=easp[0:nd, :],
                                        in1=ebsp[0:nd, :], op=mybir.AluOpType.max)

            # ---- remaining SBUF constants (needed from E1 on) ----
            w2_sb = const.tile([P, KC2, C], bf16)
            nc.sync.dma_start(out=w2_sb[:], in_=d_w2.ap().rearrange("(kc p) n -> p kc n", p=P))
            a2_sb = const.tile([P, KC2, 2], bf16)
            nc.sync.dma_start(out=a2_sb[:], in_=d_a2.ap().rearrange("(kc p) n -> p kc n", p=P))
            wc_sb = const.tile([P, OUT], f32)
            nc.sync.dma_start(out=wc_sb[:], in_=d_wc[:, :])
            bcb_sb = const.tile([G, OUT], f32)
            nc.sync.dma_start(out=bcb_sb[:], in_=d_bcb[:, :])
            b1b_sb = const.tile([P, 512], f32)
            nc.sync.dma_start(out=b1b_sb[:], in_=d_b1b[:, :])
            b2b_sb = const.tile([P, C], f32)
            nc.sync.dma_start(out=b2b_sb[:], in_=d_b2b[:, :])
            srci_sb = const.tile([P, ncols], i16)
            nc.sync.dma_start(out=srci_sb[:], in_=d_srci[:, :])
            poolm_sb = const.tile([P, nblk, G], f32)
            nc.sync.dma_start(out=poolm_sb[:], in_=d_poolm.ap())
            ident = const.tile([P, P], bf16)
            make_identity(nc, ident[:])

            def emit_d2(j):
                nd2 = min(P, nloc - j * P)
                o1b_j, _ = o1bs[j]
                for kc in range(KC2):
                    tp = psT.tile([P, P], bf16, tag="tp")
                    nc.tensor.transpose(out=tp[:, 0:nd2],
                                        in_=o1b_j[0:nd2, kc * P:(kc + 1) * P],
                                        identity=ident[0:nd2, 0:nd2])
                    nc.scalar.activation(h1T_sb[:, kc, j * P: j * P + nd2],
                                         tp[:, 0:nd2],
                                         mybir.ActivationFunctionType.Copy)
                p2 = psU.tile([P, C], f32, tag="U")
                p2a = psD.tile([P, 2], f32, tag="den")
                for kc in range(KC2):
                    lhs = h1T_sb[:, kc, j * P: j * P + nd2]
                    nc.tensor.matmul(out=p2[0:nd2, :], lhsT=lhs, rhs=w2_sb[:, kc, :],
                                     start=(kc == 0), stop=(kc == KC2 - 1))
                    nc.tensor.matmul(out=p2a[0:nd2, :], lhsT=lhs, rhs=a2_sb[:, kc, :],
                                     start=(kc == 0), stop=(kc == KC2 - 1))
                r2 = rows.tile([P, ROW2], bf16, tag="r2", name="r2")
                nc.gpsimd.memset(r2[0:nd2, C + 1:ROW2], 0.0)
                nc.scalar.activation(r2[0:nd2, 0:C], p2[0:nd2, :],
                                     mybir.ActivationFunctionType.Copy)
                nc.scalar.activation(r2[0:nd2, C:C + 1], p2a[0:nd2, 0:1],
                                     mybir.ActivationFunctionType.Copy)
                nc.sync.dma_start(out=h2_in[j * P: j * P + nd2, :], in_=r2[0:nd2, :])
                nc.scalar.activation(ald2_sb[0:nd2, j, :], p2a[0:nd2, 1:2],
                                     mybir.ActivationFunctionType.Copy)

            o1bs = {}
            # ========== E1 (+ D2 of block b-1, whose deps are ready) ========
            for b in range(nblk if do_e1 else 0):
                nd = min(P, nloc - b * P)
                t_b = tb[b]
                to = toffs[b]
                nidx = t_b * P
                cb = to * 8
                hg = work.tile([P, t_max, ROW1], bf16, tag="hg", bufs=5)
                nc.gpsimd.dma_gather(hg[:, 0:t_b, :], h_pack[:, :],
                                     srci_sb[:, cb:cb + t_b * 8],
                                     nidx, nidx, ROW1, single_packet=False)
                segc = work.tile([P, t_max, P], bf16, tag="segc", bufs=3)
                nc.sync.dma_start(
                    out=segc[:, 0:t_b, :],
                    in_=d_seg.ap().rearrange("p (t q) -> p t q", q=P)[:, to:to + t_b, :])

                if do_d2 and b > 0:
                    emit_d2(b - 1)

                # self-loop row block: local shard rows (contiguous)
                hself = work.tile([P, 512], bf16, tag="hself")
                nc.sync.dma_start(out=hself[0:nd, :],
                                  in_=h_shard[b * P: b * P + nd, 0:512])

                s32 = work.tile([P, t_max, 4], f32, tag="s32")
                nc.vector.tensor_tensor(out=s32[:, 0:t_b, :],
                                        in0=hg[:, 0:t_b, 512:516],
                                        in1=pald_all[:, to:to + t_b, :],
                                        op=mybir.AluOpType.add)
                ea = work.tile([P, t_max, 4], f32, tag="ea")
                nc.scalar.activation(ea[:, 0:t_b, :], s32[:, 0:t_b, :],
                                     mybir.ActivationFunctionType.Exp)
                eb = work.tile([P, t_max, 4], f32, tag="eb")
                nc.scalar.activation(eb[:, 0:t_b, :], s32[:, 0:t_b, :],
                                     mybir.ActivationFunctionType.Exp,
                                     scale=NEG_SLOPE)
                pbf = work.tile([P, t_max, 4], bf16, tag="pbf")
                nc.vector.tensor_tensor(out=pbf[:, 0:t_b, :], in0=ea[:, 0:t_b, :],
                                        in1=eb[:, 0:t_b, :], op=mybir.AluOpType.max)

                th = t_b // 2
                hgA = hg[:, 0:th, 0:512].rearrange("p t (h c) -> p t h c", h=H1)
                nc.vector.tensor_tensor(
                    out=hgA, in0=hgA,
                    in1=pbf[:, 0:th, :].unsqueeze(3).to_broadcast([P, th, H1, C]),
                    op=mybir.AluOpType.mult)
                hgB = hg[:, th:t_b, 0:512].rearrange("p t (h c) -> p t h c", h=H1)
                nc.vector.tensor_tensor(
                    out=hgB, in0=hgB,
                    in1=pbf[:, th:t_b, :].unsqueeze(3).to_broadcast([P, t_b - th, H1, C]),
                    op=mybir.AluOpType.mult)

                den = psD.tile([P, 4], f32, tag="den")
                for t in range(t_b):
                    nc.tensor.matmul(out=den[:, :], lhsT=segc[:, t, :],
                                     rhs=pbf[:, t, :],
                                     start=(t == 0), stop=(t == t_b - 1))
                U = psU.tile([P, 512], f32, tag="U")
                for t in range(t_b):
                    nc.tensor.matmul(out=U[:, :], lhsT=segc[:, t, :],
                                     rhs=hg[:, t, 0:512],
                                     start=(t == 0), stop=(t == t_b - 1))

                # add self-loop contribution, normalize, bias, relu
                o1 = work.tile([P, 512], f32, tag="o1")
                nc.vector.tensor_tensor(
                    out=o1[0:nd, :].rearrange("p (h c) -> p h c", h=H1),
                    in0=hself[0:nd, :].rearrange("p (h c) -> p h c", h=H1),
                    in1=pself_all[0:nd, b, :].unsqueeze(2).to_broadcast([nd, H1, C]),
                    op=mybir.AluOpType.mult)
                dtot = work.tile([P, 4], f32, tag="dtot")
                nc.vector.tensor_tensor(out=dtot[0:nd, :], in0=den[0:nd, :],
                                        in1=pself_all[0:nd, b, :],
                                        op=mybir.AluOpType.add)
                rec = work.tile([P, 4], f32, tag="rec")
                nc.vector.reciprocal(rec[0:nd, :], dtot[0:nd, :])
                nc.vector.tensor_tensor(out=o1[0:nd, :], in0=U[0:nd, :],
                                        in1=o1[0:nd, :], op=mybir.AluOpType.add)
                o1h = o1[0:nd, :].rearrange("p (h c) -> p h c", h=H1)
                nc.vector.tensor_tensor(
                    out=o1h, in0=o1h,
                    in1=rec[0:nd, :].unsqueeze(2).to_broadcast([nd, H1, C]),
                    op=mybir.AluOpType.mult)
                nc.vector.tensor_tensor(out=o1[0:nd, :], in0=o1[0:nd, :],
                                        in1=b1b_sb[0:nd, :], op=mybir.AluOpType.add)
                o1b = work.tile([P, 512], bf16, tag="o1b", bufs=2)
                nc.scalar.activation(o1b[0:nd, :], o1[0:nd, :],
                                     mybir.ActivationFunctionType.Relu)
                o1bs[b] = (o1b, nd)

            # ---- D2 for the final block ----
            if do_d2:
                emit_d2(nblk - 1)

            # ---- exchange layer-2 inputs ----
            if do_coll:
                nc.gpsimd.collective_compute(
                    "AllGather", mybir.AluOpType.bypass,
                    replica_groups=[list(range(n_cores))],
                    ins=[h2_in.opt()], outs=[h2_full.opt()])
            elif do_e2:
                nc.sync.dma_start(out=h2_full[0:nloc, :], in_=h2_in[:, :])

            # ---- L2 self-loop attention precompute (runs during AllGather) ----
            for b in range(nblk):
                nd = min(P, nloc - b * P)
                als2b = work.tile([P, 1], bf16, tag="als2b")
                nc.sync.dma_start(out=als2b[0:nd, :],
                                  in_=h2_in[b * P: b * P + nd, C:C + 1])
                s2selfp = work.tile([P, 1], f32, tag="s2selfp")
                nc.vector.tensor_tensor(out=s2selfp[0:nd, :], in0=als2b[0:nd, :],
                                        in1=ald2_sb[0:nd, b, :], op=mybir.AluOpType.add)
                ea2sp = work.tile([P, 1], f32, tag="ea2sp")
                nc.scalar.activation(ea2sp[0:nd, :], s2selfp[0:nd, :],
                                     mybir.ActivationFunctionType.Exp)
                eb2sp = work.tile([P, 1], f32, tag="eb2sp")
                nc.scalar.activation(eb2sp[0:nd, :], s2selfp[0:nd, :],
                                     mybir.ActivationFunctionType.Exp,
                                     scale=NEG_SLOPE)
                nc.vector.tensor_tensor(out=p2self_all[0:nd, b, :], in0=ea2sp[0:nd, :],
                                        in1=eb2sp[0:nd, :], op=mybir.AluOpType.max)

            # ---- pald2 streamed during the second AllGather ----
            with tc.tile_pool(name="segs2", bufs=2) as segs2:
                for g0 in range(0, ttot, CH):
                    nt = min(CH, ttot - g0)
                    sT2 = segs2.tile([P, CH, P], bf16, tag="sT2")
                    nc.sync.dma_start(
                        out=sT2[:, 0:nt, :],
                        in_=d_segT.ap().rearrange("p (t q) -> p t q", q=P)[:, g0:g0 + nt, :])
                    pch2 = psA.tile([P, CH, 4], f32, tag="pald")
                    for k in range(nt):
                        nc.tensor.matmul(out=pch2[:, k, 0:1], lhsT=sT2[:, k, :],
                                         rhs=ald2_sb[:, tile_blk[g0 + k], :],
                                         start=True, stop=True)
                    nc.vector.tensor_copy(out=pald2_all[:, g0:g0 + nt, :],
                                          in_=pch2[:, 0:nt, 0:1])

            # ================= E2: layer-2 edge phase =======================
            poolT = psP.tile([P, G], f32, tag="poolT")
            for b in range(nblk if do_e2 else 0):
                nd = min(P, nloc - b * P)
                t_b = tb[b]
                to = toffs[b]
                nidx = t_b * P
                cb = to * 8
                hg2 = work.tile([P, t_max, ROW2], bf16, tag="hg2", bufs=3)
                nc.gpsimd.dma_gather(hg2[:, 0:t_b, :], h2_full[:, :],
                                     srci_sb[:, cb:cb + t_b * 8],
                                     nidx, nidx, ROW2, single_packet=False)
                segc2 = work.tile([P, t_max, P], bf16, tag="segc", bufs=3)
                nc.sync.dma_start(
                    out=segc2[:, 0:t_b, :],
                    in_=d_seg.ap().rearrange("p (t q) -> p t q", q=P)[:, to:to + t_b, :])

                h2self = work.tile([P, C], bf16, tag="h2self")
                nc.sync.dma_start(out=h2self[0:nd, :],
                                  in_=h2_in[b * P: b * P + nd, 0:C])

                s2 = work.tile([P, t_max, 1], f32, tag="s2")
                nc.vector.tensor_tensor(out=s2[:, 0:t_b, :], in0=hg2[:, 0:t_b, C:C + 1],
                                        in1=pald2_all[:, to:to + t_b, :],
                                        op=mybir.AluOpType.add)
                ea2 = work.tile([P, t_max, 1], f32, tag="ea2")
                nc.scalar.activation(ea2[:, 0:t_b, :], s2[:, 0:t_b, :],
                                     mybir.ActivationFunctionType.Exp)
                eb2 = work.tile([P, t_max, 1], f32, tag="eb2")
                nc.scalar.activation(eb2[:, 0:t_b, :], s2[:, 0:t_b, :],
                                     mybir.ActivationFunctionType.Exp,
                                     scale=NEG_SLOPE)
                pbf2 = work.tile([P, t_max, 1], bf16, tag="pbf2")
                nc.vector.tensor_tensor(out=pbf2[:, 0:t_b, :], in0=ea2[:, 0:t_b, :],
                                        in1=eb2[:, 0:t_b, :], op=mybir.AluOpType.max)

                nc.vector.tensor_tensor(
                    out=hg2[:, 0:t_b, 0:C], in0=hg2[:, 0:t_b, 0:C],
                    in1=pbf2[:, 0:t_b, 0:1].to_broadcast([P, t_b, C]),
                    op=mybir.AluOpType.mult)

                den2 = psD.tile([P, 1], f32, tag="den")
                for t in range(t_b):
                    nc.tensor.matmul(out=den2[:, :], lhsT=segc2[:, t, :],
                                     rhs=pbf2[:, t, :],
                                     start=(t == 0), stop=(t == t_b - 1))
                U2 = psU.tile([P, C], f32, tag="U")
                for t in range(t_b):
                    nc.tensor.matmul(out=U2[:, :], lhsT=segc2[:, t, :],
                                     rhs=hg2[:, t, 0:C],
                                     start=(t == 0), stop=(t == t_b - 1))

                o2self = work.tile([P, C], f32, tag="o2self")
                nc.vector.tensor_tensor(out=o2self[0:nd, :], in0=h2self[0:nd, 0:C],
                                        in1=p2self_all[0:nd, b, :].to_broadcast([nd, C]),
                                        op=mybir.AluOpType.mult)
                d2tot = work.tile([P, 1], f32, tag="d2tot")
                nc.vector.tensor_tensor(out=d2tot[0:nd, :], in0=den2[0:nd, :],
                                        in1=p2self_all[0:nd, b, :],
                                        op=mybir.AluOpType.add)
                rec2 = work.tile([P, 1], f32, tag="rec2")
                nc.vector.reciprocal(rec2[0:nd, :], d2tot[0:nd, :])
                o2 = work.tile([P, C], f32, tag="o2")
                nc.vector.tensor_tensor(out=o2[0:nd, :], in0=U2[0:nd, :],
                                        in1=o2self[0:nd, :], op=mybir.AluOpType.add)
                nc.vector.tensor_tensor(out=o2[0:nd, :], in0=o2[0:nd, :],
                                        in1=rec2[0:nd, 0:1].to_broadcast([nd, C]),
                                        op=mybir.AluOpType.mult)
                nc.vector.tensor_tensor(out=o2[0:nd, :], in0=o2[0:nd, :],
                                        in1=b2b_sb[0:nd, :], op=mybir.AluOpType.add)
                o2r = work.tile([P, C], f32, tag="o2r")
                nc.scalar.activation(o2r[0:nd, :], o2[0:nd, :],
                                     mybir.ActivationFunctionType.Relu)
                nc.tensor.matmul(out=poolT[:, :], lhsT=o2r[0:nd, :],
                                 rhs=poolm_sb[0:nd, b, :],
                                 start=(b == 0), stop=(b == nblk - 1))

            # ================= tail: pool exchange + classifier =============
            if not do_e2:
                dummy = work.tile([G, OUT], f32, tag="dummy")
                nc.vector.tensor_copy(out=dummy[:], in_=bcb_sb[:])
                nc.sync.dma_start(out=d_out[:, :], in_=dummy[:])
            else:
              poolT_sb = work.tile([P, G], f32, tag="poolT_sb")
              nc.scalar.activation(poolT_sb[:], poolT[:, :],
                                   mybir.ActivationFunctionType.Copy)
              nc.sync.dma_start(out=pool_in[:, :], in_=poolT_sb[:])
              if do_coll:
                  nc.gpsimd.collective_compute(
                      "AllReduce", mybir.AluOpType.add,
                      replica_groups=[list(range(n_cores))],
                      ins=[pool_in.opt()], outs=[pool_out.opt()])
              else:
                  nc.sync.dma_start(out=pool_out[:, :], in_=pool_in[:, :])
              poolF_sb = work.tile([P, G], f32, tag="poolF_sb")
              nc.sync.dma_start(out=poolF_sb[:], in_=pool_out[:, :])
              ofin = psD.tile([G, OUT], f32, tag="den")
              nc.tensor.matmul(out=ofin[:, :], lhsT=poolF_sb[:], rhs=wc_sb[:],
                               start=True, stop=True)
              ofin_sb = work.tile([G, OUT], f32, tag="ofin_sb")
              nc.vector.tensor_tensor(out=ofin_sb[:], in0=ofin[:, :], in1=bcb_sb[:],
                                      op=mybir.AluOpType.add)
              nc.sync.dma_start(out=d_out[:, :], in_=ofin_sb[:])

    nc.compile()
    return nc


# ------------------------------------------------------------------
#  runner
# ------------------------------------------------------------------

_CACHE = {}


def _get_nc(meta):
    key = (meta['n_cores'], meta['nblk'], meta['tb'], meta['nloc'])
    if key not in _CACHE:
        _CACHE[key] = _build(meta)
    return _CACHE[key]


def _in_maps(common, per_core):
    maps = []
    for pc in per_core:
        m = dict(common)
        m.update(pc)
        maps.append(m)
    return maps


def kernel(**inputs) -> np.ndarray:
    common, per_core, meta = _prep(**inputs)
    nc = _get_nc(meta)
    from concourse.bass_utils import run_bass_kernel_spmd
    res = run_bass_kernel_spmd(nc, _in_maps(common, per_core),
                               core_ids=list(range(meta['n_cores'])))
    return np.asarray(res.results[0]['out'], np.float32).reshape(-1)
